# revision 1
# baseline (speedup 1.0000x reference)
"""Trainium2 Bass kernel: batched 2D DCT-II (unnormalized), x: (8, 2048, 2048) f32.

Math: per image X, the unnormalized 2D DCT-II is Z = C @ X @ C^T with
C[k,n] = cos(pi*(2n+1)*k/(2N)).  Let D = C^T.  Using the PE's
out = lhsT.T @ rhs semantics with the *data* as the stationary operand and D as
the moving operand, the two passes chain with no transposes:
    pass 1:  T = X^T @ D      (lhsT = X tiles,  rhs = D)   -> T[c, f]
    pass 2:  Z = T^T @ D      (lhsT = T tiles,  rhs = D)   -> Z = D^T X D = C X C^T

Sharding: batch dim 8 -> one image per NeuronCore (data parallel, no comms).

Dtype modes:
  "f32r"  - single fp32r matmul per term (full PE rate; TF32-like 11-bit
            mantissa operand rounding; ~2e-4 relative-to-absmax error).
  "split" - hi/lo bf16 decomposition, 3 matmuls per term (~5e-6 error, 3x cost).
"""

import numpy as np
from contextlib import ExitStack

import concourse.bass as bass
import concourse.bacc as bacc
import concourse.tile as tile
from concourse import mybir
from concourse.bass_utils import run_bass_kernel_spmd

F32 = mybir.dt.float32
F32R = mybir.dt.float32r
BF16 = mybir.dt.bfloat16

import os

MODE = os.environ.get("DCT_MODE", "bfly")  # "bfly", "f32r", or "split"

B = 8          # batch == n_cores
N = 2048       # image is N x N
P = 128        # partitions
KT = N // P    # 16 k-tiles along any contraction
FC = 512       # chunk width (pass-1 f-chunk, pass-2 g-chunk, PSUM bank)
NFC = N // FC  # 4 chunks
H = N // 2     # butterfly half size
KT2 = H // P   # 8 k-tiles at half contraction


def _round_f32r(a: np.ndarray) -> np.ndarray:
    """fp32r = round-to-nearest, 11 explicit mantissa bits (drop low 12)."""
    b = np.ascontiguousarray(a, dtype=np.float32).view(np.uint32)
    r = ((b + np.uint32(0x800)) & np.uint32(0xFFFFF000)).view(np.float32)
    return r


def _split_bf16(a: np.ndarray):
    import ml_dtypes

    hi = a.astype(ml_dtypes.bfloat16)
    lo = (a - hi.astype(np.float32)).astype(ml_dtypes.bfloat16)
    return hi, lo


def _dct_matrix_d() -> np.ndarray:
    # D[n, k] = cos(pi * (2n+1) * k / (2N)), exact in float64
    n = np.arange(N, dtype=np.float64)[:, None]
    k = np.arange(N, dtype=np.float64)[None, :]
    d = np.cos(np.pi * (2.0 * n + 1.0) * k / (2.0 * N))
    return d.astype(np.float32)


def _build_f32r() -> bass.Bass:
    """fp32r two-pass DCT with the intermediate T round-tripped via DRAM.

    Pass 1 streams X once (one column-block per chain, all 4 f-chunks while
    the block is resident).  T chunks are written back to a DRAM scratch and
    re-streamed as pass-2 stationary tiles.  D stays resident in SBUF.
    """
    nc = bacc.Bacc(None, target_bir_lowering=False)
    x_ext = nc.declare_dram_parameter("x", [N, N], F32R, isOutput=False)
    d_ext = nc.declare_dram_parameter("d", [N, N], F32R, isOutput=False)
    z_ext = nc.declare_dram_parameter("z", [N, N], F32, isOutput=True)

    with ExitStack() as ctx:
        tc = ctx.enter_context(tile.TileContext(nc))
        d_pool = ctx.enter_context(tc.tile_pool(name="d", bufs=1))
        x_pool = ctx.enter_context(tc.tile_pool(name="x", bufs=3))
        t_pool = ctx.enter_context(tc.tile_pool(name="t", bufs=6))
        z_pool = ctx.enter_context(tc.tile_pool(name="z", bufs=3))
        dram = ctx.enter_context(tc.tile_pool(name="dram", bufs=1, space="DRAM"))
        ps1 = ctx.enter_context(tc.tile_pool(name="ps1", bufs=4, space="PSUM"))
        ps2 = ctx.enter_context(tc.tile_pool(name="ps2", bufs=4, space="PSUM"))

        t_dram = dram.tile([N, N], F32R, name="t_dram")

        # First column-block of X loads before D so pass 1 starts early.
        d_sb = [
            d_pool.tile([P, N], F32R, tag=f"d{t}", name=f"d{t}") for t in range(KT)
        ]

        def load_x(cb):
            xt = x_pool.tile([P, N], F32R, tag="x", name="xt")
            nc.sync.dma_start(
                xt[:].rearrange("p (t m) -> p t m", t=KT),
                x_ext[:, cb * P : (cb + 1) * P].rearrange("(t p) m -> p t m", p=P),
            )
            return xt

        x0 = load_x(0)
        # D f-chunk 0 for all 16 row-tiles (pass-1 chain 0 needs only these)
        for fcol in range(NFC):
            for t in range(KT):
                nc.sync.dma_start(
                    d_sb[t][:, fcol * FC : (fcol + 1) * FC],
                    d_ext[t * P : (t + 1) * P, fcol * FC : (fcol + 1) * FC],
                )
            if fcol == 0:
                # remaining D chunks stream behind pass-1 compute
                pass

        # ---- pass 1: per column-block cb, all f-chunks: T[cb,:] = (X^T D)[cb,:]
        for cb in range(KT):
            xt = x0 if cb == 0 else load_x(cb)
            for fc in range(NFC):
                pt = ps1.tile([P, FC], F32, tag="ps1", name="pt")
                for rt in range(KT):
                    nc.tensor.matmul(
                        pt[:],
                        lhsT=xt[:, rt * P : (rt + 1) * P],
                        rhs=d_sb[rt][:, fc * FC : (fc + 1) * FC],
                        start=(rt == 0),
                        stop=(rt == KT - 1),
                    )
                tt = t_pool.tile([P, FC], F32R, tag="t", name="tt")
                nc.vector.tensor_copy(tt[:], pt[:])
                nc.scalar.dma_start(
                    t_dram[cb * P : (cb + 1) * P, fc * FC : (fc + 1) * FC], tt[:]
                )

        # ---- pass 2: per f-block fb: Z[fb,:] = (T^T D)[fb,:]
        for fb in range(KT):
            tf = x_pool.tile([P, N], F32R, tag="x", name="tf")
            nc.sync.dma_start(
                tf[:].rearrange("p (t m) -> p t m", t=KT),
                t_dram[:, fb * P : (fb + 1) * P].rearrange("(t p) m -> p t m", p=P),
            )
            for g in range(NFC):
                pz = ps2.tile([P, FC], F32, tag="ps2", name="pz")
                for ct in range(KT):
                    nc.tensor.matmul(
                        pz[:],
                        lhsT=tf[:, ct * P : (ct + 1) * P],
                        rhs=d_sb[ct][:, g * FC : (g + 1) * FC],
                        start=(ct == 0),
                        stop=(ct == KT - 1),
                    )
                zt = z_pool.tile([P, FC], F32, tag="z", name="zt")
                nc.vector.tensor_copy(zt[:], pz[:])
                nc.scalar.dma_start(
                    z_ext[fb * P : (fb + 1) * P, g * FC : (g + 1) * FC], zt[:]
                )

    nc.finalize()
    return nc


def _build_bfly() -> bass.Bass:
    """Radix-2 even/odd DCT factorization in fp32r: each 1D DCT-II of size N
    becomes two size-N/2 cosine transforms of the folded sequences
    u = x_top + reverse(x_bot), v = x_top - reverse(x_bot):
        y[2j]   = sum_{n<H} u[n] De[n, j],   De[n,j] = cos(pi (2n+1) j / N)
        y[2j+1] = sum_{n<H} v[n] Do[n, j],   Do[n,j] = cos(pi (2n+1)(2j+1) / 2N)
    Halves the matmul work per pass.  Pass-1 folding is done on the host
    (u/v uploaded); pass-2 folding of the intermediate T is done by DVE with a
    reversed-row DMA load.  Outputs are de-interleaved on chip (strided DVE
    writes) + stride-2-row DMA stores, so all DRAM traffic stays contiguous
    per partition.
    """
    nc = bacc.Bacc(None, target_bir_lowering=False)
    u_ext = nc.declare_dram_parameter("u", [H, N], F32R, isOutput=False)
    v_ext = nc.declare_dram_parameter("v", [H, N], F32R, isOutput=False)
    de_ext = nc.declare_dram_parameter("de", [H, H], F32R, isOutput=False)
    do_ext = nc.declare_dram_parameter("do", [H, H], F32R, isOutput=False)
    z_ext = nc.declare_dram_parameter("z", [N, N], F32, isOutput=True)

    with ExitStack() as ctx:
        tc = ctx.enter_context(tile.TileContext(nc))
        d_pool = ctx.enter_context(tc.tile_pool(name="d", bufs=1))
        x_pool = ctx.enter_context(tc.tile_pool(name="x", bufs=4))
        t_pool = ctx.enter_context(tc.tile_pool(name="t", bufs=4))
        b_pool = ctx.enter_context(tc.tile_pool(name="b", bufs=4))
        z_pool = ctx.enter_context(tc.tile_pool(name="z", bufs=3))
        dram = ctx.enter_context(tc.tile_pool(name="dram", bufs=1, space="DRAM"))
        # PSUM: pass-1 accumulators 2x1 bank; pass-2 output chains share one
        # 6-slot pool (6 banks) so slow de-interleave copies never stall the
        # next chain. 8 banks total (no reversal matmuls anymore).
        ps1 = ctx.enter_context(tc.tile_pool(name="ps1", bufs=2, space="PSUM"))
        ps2 = ctx.enter_context(tc.tile_pool(name="ps2", bufs=6, space="PSUM"))

        # T in blocked layout: cols [0,H) = even outputs, [H,2H) = odd
        t_dram = dram.tile([N, N], F32R, name="t_dram")

        de_sb = [
            d_pool.tile([P, H], F32R, tag=f"de{t}", name=f"de{t}")
            for t in range(KT2)
        ]
        do_sb = [
            d_pool.tile([P, H], F32R, tag=f"do{t}", name=f"do{t}")
            for t in range(KT2)
        ]

        def load_block(ext, cb, tag):
            w = x_pool.tile([P, H], F32R, tag=tag, name="w_" + tag)
            nc.sync.dma_start(
                w[:].rearrange("p (t m) -> p t m", t=KT2),
                ext[:, cb * P : (cb + 1) * P].rearrange("(t p) m -> p t m", p=P),
            )
            return w

        # loads in exact first-consumption order: u0, de jc0, de jc1, v0,
        # do jc0, do jc1; the pass-2 reversal matrix r last.
        u0 = load_block(u_ext, 0, "u")
        for jc in range(2):
            for t in range(KT2):
                nc.sync.dma_start(
                    de_sb[t][:, jc * FC : (jc + 1) * FC],
                    de_ext[t * P : (t + 1) * P, jc * FC : (jc + 1) * FC],
                )
        v0 = load_block(v_ext, 0, "v")
        for jc in range(2):
            for t in range(KT2):
                nc.sync.dma_start(
                    do_sb[t][:, jc * FC : (jc + 1) * FC],
                    do_ext[t * P : (t + 1) * P, jc * FC : (jc + 1) * FC],
                )

        # ---- pass 1: T_blk[cb, :] ----
        for cb in range(KT):
            ut = u0 if cb == 0 else load_block(u_ext, cb, "u")
            vt = v0 if cb == 0 else load_block(v_ext, cb, "v")
            for half, (wt, dsb) in enumerate(((ut, de_sb), (vt, do_sb))):
                for jc in range(2):
                    pt = ps1.tile([P, FC], F32, tag="acc", name="pt")
                    for rt in range(KT2):
                        nc.tensor.matmul(
                            pt[:],
                            lhsT=wt[:, rt * P : (rt + 1) * P],
                            rhs=dsb[rt][:, jc * FC : (jc + 1) * FC],
                            start=(rt == 0),
                            stop=(rt == KT2 - 1),
                        )
                    tt = t_pool.tile([P, FC], F32R, tag="t", name="tt")
                    nc.vector.tensor_copy(tt[:], pt[:])
                    col0 = half * H + jc * FC
                    # Bottom-half blocks (cb>=8) arrive partition-reversed
                    # (host reversed their lhsT columns) and are stored
                    # mirror-ordered: row 1024+k holds T[2047-k].  Then the
                    # pass-2 fold reads both halves with plain ascending loads.
                    row0 = cb * P if cb < KT2 else (23 * P - cb * P)
                    nc.scalar.dma_start(
                        t_dram[row0 : row0 + P, col0 : col0 + FC], tt[:]
                    )

        # ---- pass 2: fold T over rows, transform, de-interleave out ----
        # bot_rev[c', f] = T[2047-c', f]: partition reversal via one PE matmul
        # with the reversal permutation R (out[m,n] = bot[127-m, n]); the
        # tile-order flip (ct -> 7-ct) via a reversed free-dim view in the add.
        # Software-pipelined: loads run 3 blocks ahead, reversal matmul + DVE
        # fold 2 ahead, so block fb's chains never wait on its fold.
        folded: dict = {}

        def p2_load(fb):
            top = b_pool.tile([P, H], F32R, tag="top", name="top")
            nc.sync.dma_start(
                top[:].rearrange("p (t m) -> p t m", t=KT2),
                t_dram[0:H, fb * P : (fb + 1) * P].rearrange(
                    "(t p) m -> p t m", p=P
                ),
            )
            bot = b_pool.tile([P, H], F32R, tag="bot", name="bot")
            nc.sync.dma_start(
                bot[:].rearrange("p (t m) -> p t m", t=KT2),
                t_dram[H:N, fb * P : (fb + 1) * P].rearrange(
                    "(t p) m -> p t m", p=P
                ),
            )
            folded[fb] = (top, bot)

        def p2_fold(fb):
            # mirror-ordered bottom storage makes the fold a plain 2D add/sub
            top, bot = folded[fb]
            u2 = b_pool.tile([P, H], F32R, tag="u2", name="u2")
            nc.vector.tensor_add(u2[:], top[:], bot[:])
            v2 = b_pool.tile([P, H], F32R, tag="v2", name="v2")
            nc.vector.tensor_sub(v2[:], top[:], bot[:])
            folded[fb] = (u2, v2)

        p2_load(0)
        p2_load(1)
        p2_fold(0)
        p2_load(2)
        p2_fold(1)
        for fb in range(KT):
            u2, v2 = folded.pop(fb)
            # f_blk block fb -> actual Z rows (de-interleave rows via stride 2)
            if fb < KT2:
                row0 = 2 * fb * P
                row_stop = row0 + 2 * P
            else:
                row0 = 2 * (fb - KT2) * P + 1
                row_stop = row0 + 2 * P - 1
            for jc in range(2):
                pe_ = ps2.tile([P, FC], F32, tag="zacc", name="pe_")
                for ct in range(KT2):
                    nc.tensor.matmul(
                        pe_[:],
                        lhsT=u2[:, ct * P : (ct + 1) * P],
                        rhs=de_sb[ct][:, jc * FC : (jc + 1) * FC],
                        start=(ct == 0),
                        stop=(ct == KT2 - 1),
                    )
                po_ = ps2.tile([P, FC], F32, tag="zacc", name="po_")
                for ct in range(KT2):
                    nc.tensor.matmul(
                        po_[:],
                        lhsT=v2[:, ct * P : (ct + 1) * P],
                        rhs=do_sb[ct][:, jc * FC : (jc + 1) * FC],
                        start=(ct == 0),
                        stop=(ct == KT2 - 1),
                    )
                zt = z_pool.tile([P, 2 * FC], F32, tag="z", name="zt")
                nc.scalar.copy(zt[:, 0 : 2 * FC : 2], pe_[:])
                nc.vector.tensor_copy(zt[:, 1 : 2 * FC : 2], po_[:])
                nc.scalar.dma_start(
                    z_ext[row0:row_stop:2, jc * 2 * FC : (jc + 1) * 2 * FC],
                    zt[:],
                )
            if fb + 3 < KT:
                p2_load(fb + 3)
            if fb + 2 < KT:
                p2_fold(fb + 2)

    nc.finalize()
    return nc


def _build_split() -> bass.Bass:
    """hi/lo bf16 decomposition: each logical matmul = 3 bf16 matmuls
    (Xh Dh + Xh Dl + Xl Dh), accumulated in the same PSUM chain."""
    nc = bacc.Bacc(None, target_bir_lowering=False)
    xh_ext = nc.declare_dram_parameter("xh", [N, N], BF16, isOutput=False)
    xl_ext = nc.declare_dram_parameter("xl", [N, N], BF16, isOutput=False)
    dh_ext = nc.declare_dram_parameter("dh", [N, N], BF16, isOutput=False)
    dl_ext = nc.declare_dram_parameter("dl", [N, N], BF16, isOutput=False)
    z_ext = nc.declare_dram_parameter("z", [N, N], F32, isOutput=True)

    with ExitStack() as ctx:
        tc = ctx.enter_context(tile.TileContext(nc))
        d_pool = ctx.enter_context(tc.tile_pool(name="d", bufs=1))
        x_pool = ctx.enter_context(tc.tile_pool(name="x", bufs=3))
        w_pool = ctx.enter_context(tc.tile_pool(name="w", bufs=3))
        t_pool = ctx.enter_context(tc.tile_pool(name="t", bufs=KT))
        z_pool = ctx.enter_context(tc.tile_pool(name="z", bufs=3))
        ps1 = ctx.enter_context(tc.tile_pool(name="ps1", bufs=4, space="PSUM"))
        ps2 = ctx.enter_context(tc.tile_pool(name="ps2", bufs=4, space="PSUM"))

        dh_sb = [
            d_pool.tile([P, N], BF16, tag=f"dh{t}", name=f"dh{t}")
            for t in range(KT)
        ]
        dl_sb = [
            d_pool.tile([P, N], BF16, tag=f"dl{t}", name=f"dl{t}")
            for t in range(KT)
        ]
        for fcol in range(NFC):
            for t in range(KT):
                nc.sync.dma_start(
                    dh_sb[t][:, fcol * FC : (fcol + 1) * FC],
                    dh_ext[t * P : (t + 1) * P, fcol * FC : (fcol + 1) * FC],
                )
                nc.sync.dma_start(
                    dl_sb[t][:, fcol * FC : (fcol + 1) * FC],
                    dl_ext[t * P : (t + 1) * P, fcol * FC : (fcol + 1) * FC],
                )

        for fc in range(NFC):
            t_tiles = []
            for cb in range(KT):
                xht = x_pool.tile([P, N], BF16, tag="xh", name="xht")
                xlt = x_pool.tile([P, N], BF16, tag="xl", name="xlt")
                for t_, ext in ((xht, xh_ext), (xlt, xl_ext)):
                    nc.sync.dma_start(
                        t_[:].rearrange("p (t m) -> p t m", t=KT),
                        ext[:, cb * P : (cb + 1) * P].rearrange(
                            "(t p) m -> p t m", p=P
                        ),
                    )
                pt = ps1.tile([P, FC], F32, tag="ps1", name="pt")
                nmm = 3 * KT
                i = 0
                for rt in range(KT):
                    dh = dh_sb[rt][:, fc * FC : (fc + 1) * FC]
                    dl = dl_sb[rt][:, fc * FC : (fc + 1) * FC]
                    xh = xht[:, rt * P : (rt + 1) * P]
                    xl = xlt[:, rt * P : (rt + 1) * P]
                    for l_, r_ in ((xh, dh), (xh, dl), (xl, dh)):
                        nc.tensor.matmul(
                            pt[:], lhsT=l_, rhs=r_,
                            start=(i == 0), stop=(i == nmm - 1),
                        )
                        i += 1
                # split T on device: th = bf16(T), tl = bf16(T - th)
                th = t_pool.tile([P, FC], BF16, tag="th", name="th")
                tl = t_pool.tile([P, FC], BF16, tag="tl", name="tl")
                tmp = w_pool.tile([P, FC], F32, tag="tmp", name="tmp")
                nc.vector.tensor_copy(th[:], pt[:])
                nc.scalar.copy(tmp[:], th[:])
                nc.vector.tensor_sub(tmp[:], pt[:], tmp[:])
                nc.vector.tensor_copy(tl[:], tmp[:])
                t_tiles.append((th, tl))

            for fb in range(FC // P):
                for g in range(NFC):
                    pz = ps2.tile([P, FC], F32, tag="ps2", name="pz")
                    nmm = 3 * KT
                    i = 0
                    for ct in range(KT):
                        th, tl = t_tiles[ct]
                        dh = dh_sb[ct][:, g * FC : (g + 1) * FC]
                        dl = dl_sb[ct][:, g * FC : (g + 1) * FC]
                        thb = th[:, fb * P : (fb + 1) * P]
                        tlb = tl[:, fb * P : (fb + 1) * P]
                        for l_, r_ in ((thb, dh), (thb, dl), (tlb, dh)):
                            nc.tensor.matmul(
                                pz[:], lhsT=l_, rhs=r_,
                                start=(i == 0), stop=(i == nmm - 1),
                            )
                            i += 1
                    zt = z_pool.tile([P, FC], F32, tag="z", name="zt")
                    nc.vector.tensor_copy(zt[:], pz[:])
                    row0 = (fc * (FC // P) + fb) * P
                    nc.sync.dma_start(
                        z_ext[row0 : row0 + P, g * FC : (g + 1) * FC], zt[:]
                    )

    nc.finalize()
    return nc


_PROGRAM_CACHE: dict = {}


_BUILDERS = {"f32r": _build_f32r, "bfly": _build_bfly, "split": _build_split}


def _get_program(mode: str) -> bass.Bass:
    if mode not in _PROGRAM_CACHE:
        _PROGRAM_CACHE[mode] = _BUILDERS[mode]()
    return _PROGRAM_CACHE[mode]


def _make_in_maps(x: np.ndarray, mode: str):
    if mode == "f32r":
        dr = _round_f32r(_dct_matrix_d())
        return [{"x": _round_f32r(x[i]), "d": dr} for i in range(B)]
    if mode == "bfly":
        n2 = np.arange(H, dtype=np.float64)[:, None]
        j2 = np.arange(H, dtype=np.float64)[None, :]
        de = _round_f32r(np.cos(np.pi * (2 * n2 + 1) * j2 / N).astype(np.float32))
        do = _round_f32r(
            np.cos(np.pi * (2 * n2 + 1) * (2 * j2 + 1) / (2 * N)).astype(
                np.float32
            )
        )
        maps = []
        for i in range(B):
            xi = np.asarray(x[i], dtype=np.float32)
            xr = xi[::-1]
            u = _round_f32r(xi[:H] + xr[:H])
            v = _round_f32r(xi[:H] - xr[:H])
            # Column-reverse blocks 8..15: pass-1 output partitions for those
            # blocks then come out mirror-ordered, which makes the pass-2
            # bottom-half fold a plain ascending read (see _build_bfly).
            for w in (u, v):
                blk = w[:, H:].reshape(H, KT2, P)
                w[:, H:] = blk[:, :, ::-1].reshape(H, H)
            maps.append({"u": u, "v": v, "de": de, "do": do})
        return maps
    dh, dl = _split_bf16(_dct_matrix_d())
    maps = []
    for i in range(B):
        xh, xl = _split_bf16(np.ascontiguousarray(x[i], dtype=np.float32))
        maps.append({"xh": xh, "xl": xl, "dh": dh, "dl": dl})
    return maps


def kernel(x: np.ndarray) -> np.ndarray:
    x = np.asarray(x)
    assert x.shape == (B, N, N), x.shape
    nc = _get_program(MODE)
    in_maps = _make_in_maps(x, MODE)
    res = run_bass_kernel_spmd(nc, in_maps, list(range(B)))
    out = np.stack([res.results[i]["z"] for i in range(B)], axis=0)
    return out.astype(np.float32, copy=False)



# revision 5
# speedup vs baseline: 1.1706x; 1.1706x over previous
"""Trainium2 Bass kernel: batched 2D DCT-II (unnormalized), x: (8, 2048, 2048) f32.

Math: per image X, the unnormalized 2D DCT-II is Z = C @ X @ C^T with
C[k,n] = cos(pi*(2n+1)*k/(2N)).  Let D = C^T.  Using the PE's
out = lhsT.T @ rhs semantics with the *data* as the stationary operand and D as
the moving operand, the two passes chain with no transposes:
    pass 1:  T = X^T @ D      (lhsT = X tiles,  rhs = D)   -> T[c, f]
    pass 2:  Z = T^T @ D      (lhsT = T tiles,  rhs = D)   -> Z = D^T X D = C X C^T

Sharding: batch dim 8 -> one image per NeuronCore (data parallel, no comms).

Dtype modes:
  "f32r"  - single fp32r matmul per term (full PE rate; TF32-like 11-bit
            mantissa operand rounding; ~2e-4 relative-to-absmax error).
  "split" - hi/lo bf16 decomposition, 3 matmuls per term (~5e-6 error, 3x cost).
"""

import numpy as np
from contextlib import ExitStack

import concourse.bass as bass
import concourse.bacc as bacc
import concourse.tile as tile
from concourse import mybir
from concourse.bass_utils import run_bass_kernel_spmd

F32 = mybir.dt.float32
F32R = mybir.dt.float32r
BF16 = mybir.dt.bfloat16

import os

MODE = os.environ.get("DCT_MODE", "bfly16")  # "bfly16", "bfly", "f32r", "split"

B = 8          # batch == n_cores
N = 2048       # image is N x N
P = 128        # partitions
KT = N // P    # 16 k-tiles along any contraction
FC = 512       # chunk width (pass-1 f-chunk, pass-2 g-chunk, PSUM bank)
NFC = N // FC  # 4 chunks
H = N // 2     # butterfly half size
KT2 = H // P   # 8 k-tiles at half contraction


def _round_f32r(a: np.ndarray) -> np.ndarray:
    """fp32r = round-to-nearest, 11 explicit mantissa bits (drop low 12)."""
    b = np.ascontiguousarray(a, dtype=np.float32).view(np.uint32)
    r = ((b + np.uint32(0x800)) & np.uint32(0xFFFFF000)).view(np.float32)
    return r


def _split_bf16(a: np.ndarray):
    import ml_dtypes

    hi = a.astype(ml_dtypes.bfloat16)
    lo = (a - hi.astype(np.float32)).astype(ml_dtypes.bfloat16)
    return hi, lo


def _dct_matrix_d() -> np.ndarray:
    # D[n, k] = cos(pi * (2n+1) * k / (2N)), exact in float64
    n = np.arange(N, dtype=np.float64)[:, None]
    k = np.arange(N, dtype=np.float64)[None, :]
    d = np.cos(np.pi * (2.0 * n + 1.0) * k / (2.0 * N))
    return d.astype(np.float32)


def _build_f32r() -> bass.Bass:
    """fp32r two-pass DCT with the intermediate T round-tripped via DRAM.

    Pass 1 streams X once (one column-block per chain, all 4 f-chunks while
    the block is resident).  T chunks are written back to a DRAM scratch and
    re-streamed as pass-2 stationary tiles.  D stays resident in SBUF.
    """
    nc = bacc.Bacc(None, target_bir_lowering=False)
    x_ext = nc.declare_dram_parameter("x", [N, N], F32R, isOutput=False)
    d_ext = nc.declare_dram_parameter("d", [N, N], F32R, isOutput=False)
    z_ext = nc.declare_dram_parameter("z", [N, N], F32, isOutput=True)

    with ExitStack() as ctx:
        tc = ctx.enter_context(tile.TileContext(nc))
        d_pool = ctx.enter_context(tc.tile_pool(name="d", bufs=1))
        x_pool = ctx.enter_context(tc.tile_pool(name="x", bufs=3))
        t_pool = ctx.enter_context(tc.tile_pool(name="t", bufs=6))
        z_pool = ctx.enter_context(tc.tile_pool(name="z", bufs=3))
        dram = ctx.enter_context(tc.tile_pool(name="dram", bufs=1, space="DRAM"))
        ps1 = ctx.enter_context(tc.tile_pool(name="ps1", bufs=4, space="PSUM"))
        ps2 = ctx.enter_context(tc.tile_pool(name="ps2", bufs=4, space="PSUM"))

        t_dram = dram.tile([N, N], F32R, name="t_dram")

        # First column-block of X loads before D so pass 1 starts early.
        d_sb = [
            d_pool.tile([P, N], F32R, tag=f"d{t}", name=f"d{t}") for t in range(KT)
        ]

        def load_x(cb):
            xt = x_pool.tile([P, N], F32R, tag="x", name="xt")
            nc.sync.dma_start(
                xt[:].rearrange("p (t m) -> p t m", t=KT),
                x_ext[:, cb * P : (cb + 1) * P].rearrange("(t p) m -> p t m", p=P),
            )
            return xt

        x0 = load_x(0)
        # D f-chunk 0 for all 16 row-tiles (pass-1 chain 0 needs only these)
        for fcol in range(NFC):
            for t in range(KT):
                nc.sync.dma_start(
                    d_sb[t][:, fcol * FC : (fcol + 1) * FC],
                    d_ext[t * P : (t + 1) * P, fcol * FC : (fcol + 1) * FC],
                )
            if fcol == 0:
                # remaining D chunks stream behind pass-1 compute
                pass

        # ---- pass 1: per column-block cb, all f-chunks: T[cb,:] = (X^T D)[cb,:]
        for cb in range(KT):
            xt = x0 if cb == 0 else load_x(cb)
            for fc in range(NFC):
                pt = ps1.tile([P, FC], F32, tag="ps1", name="pt")
                for rt in range(KT):
                    nc.tensor.matmul(
                        pt[:],
                        lhsT=xt[:, rt * P : (rt + 1) * P],
                        rhs=d_sb[rt][:, fc * FC : (fc + 1) * FC],
                        start=(rt == 0),
                        stop=(rt == KT - 1),
                    )
                tt = t_pool.tile([P, FC], F32R, tag="t", name="tt")
                nc.vector.tensor_copy(tt[:], pt[:])
                nc.scalar.dma_start(
                    t_dram[cb * P : (cb + 1) * P, fc * FC : (fc + 1) * FC], tt[:]
                )

        # ---- pass 2: per f-block fb: Z[fb,:] = (T^T D)[fb,:]
        for fb in range(KT):
            tf = x_pool.tile([P, N], F32R, tag="x", name="tf")
            nc.sync.dma_start(
                tf[:].rearrange("p (t m) -> p t m", t=KT),
                t_dram[:, fb * P : (fb + 1) * P].rearrange("(t p) m -> p t m", p=P),
            )
            for g in range(NFC):
                pz = ps2.tile([P, FC], F32, tag="ps2", name="pz")
                for ct in range(KT):
                    nc.tensor.matmul(
                        pz[:],
                        lhsT=tf[:, ct * P : (ct + 1) * P],
                        rhs=d_sb[ct][:, g * FC : (g + 1) * FC],
                        start=(ct == 0),
                        stop=(ct == KT - 1),
                    )
                zt = z_pool.tile([P, FC], F32, tag="z", name="zt")
                nc.vector.tensor_copy(zt[:], pz[:])
                nc.scalar.dma_start(
                    z_ext[fb * P : (fb + 1) * P, g * FC : (g + 1) * FC], zt[:]
                )

    nc.finalize()
    return nc


def _build_bfly() -> bass.Bass:
    """Radix-2 even/odd DCT factorization in fp32r: each 1D DCT-II of size N
    becomes two size-N/2 cosine transforms of the folded sequences
    u = x_top + reverse(x_bot), v = x_top - reverse(x_bot):
        y[2j]   = sum_{n<H} u[n] De[n, j],   De[n,j] = cos(pi (2n+1) j / N)
        y[2j+1] = sum_{n<H} v[n] Do[n, j],   Do[n,j] = cos(pi (2n+1)(2j+1) / 2N)
    Halves the matmul work per pass.  Pass-1 folding is done on the host
    (u/v uploaded); pass-2 folding of the intermediate T is done by DVE with a
    reversed-row DMA load.  Outputs are de-interleaved on chip (strided DVE
    writes) + stride-2-row DMA stores, so all DRAM traffic stays contiguous
    per partition.
    """
    nc = bacc.Bacc(None, target_bir_lowering=False)
    u_ext = nc.declare_dram_parameter("u", [H, N], F32R, isOutput=False)
    v_ext = nc.declare_dram_parameter("v", [H, N], F32R, isOutput=False)
    de_ext = nc.declare_dram_parameter("de", [H, H], F32R, isOutput=False)
    do_ext = nc.declare_dram_parameter("do", [H, H], F32R, isOutput=False)
    z_ext = nc.declare_dram_parameter("z", [N, N], F32, isOutput=True)

    with ExitStack() as ctx:
        tc = ctx.enter_context(tile.TileContext(nc))
        d_pool = ctx.enter_context(tc.tile_pool(name="d", bufs=1))
        x_pool = ctx.enter_context(tc.tile_pool(name="x", bufs=4))
        t_pool = ctx.enter_context(tc.tile_pool(name="t", bufs=4))
        b_pool = ctx.enter_context(tc.tile_pool(name="b", bufs=4))
        z_pool = ctx.enter_context(tc.tile_pool(name="z", bufs=3))
        dram = ctx.enter_context(tc.tile_pool(name="dram", bufs=1, space="DRAM"))
        # PSUM: pass-1 accumulators 2x1 bank; pass-2 output chains share one
        # 6-slot pool (6 banks) so slow de-interleave copies never stall the
        # next chain. 8 banks total (no reversal matmuls anymore).
        ps1 = ctx.enter_context(tc.tile_pool(name="ps1", bufs=2, space="PSUM"))
        ps2 = ctx.enter_context(tc.tile_pool(name="ps2", bufs=6, space="PSUM"))

        # T in blocked layout: cols [0,H) = even outputs, [H,2H) = odd
        t_dram = dram.tile([N, N], F32R, name="t_dram")

        de_sb = [
            d_pool.tile([P, H], F32R, tag=f"de{t}", name=f"de{t}")
            for t in range(KT2)
        ]
        do_sb = [
            d_pool.tile([P, H], F32R, tag=f"do{t}", name=f"do{t}")
            for t in range(KT2)
        ]

        def load_block(ext, cb, tag):
            w = x_pool.tile([P, H], F32R, tag=tag, name="w_" + tag)
            nc.sync.dma_start(
                w[:].rearrange("p (t m) -> p t m", t=KT2),
                ext[:, cb * P : (cb + 1) * P].rearrange("(t p) m -> p t m", p=P),
            )
            return w

        # loads in exact first-consumption order: u0, de jc0, de jc1, v0,
        # do jc0, do jc1; the pass-2 reversal matrix r last.
        u0 = load_block(u_ext, 0, "u")
        for jc in range(2):
            for t in range(KT2):
                nc.sync.dma_start(
                    de_sb[t][:, jc * FC : (jc + 1) * FC],
                    de_ext[t * P : (t + 1) * P, jc * FC : (jc + 1) * FC],
                )
        v0 = load_block(v_ext, 0, "v")
        for jc in range(2):
            for t in range(KT2):
                nc.sync.dma_start(
                    do_sb[t][:, jc * FC : (jc + 1) * FC],
                    do_ext[t * P : (t + 1) * P, jc * FC : (jc + 1) * FC],
                )

        # ---- pass 1: T_blk[cb, :] ----
        for cb in range(KT):
            ut = u0 if cb == 0 else load_block(u_ext, cb, "u")
            vt = v0 if cb == 0 else load_block(v_ext, cb, "v")
            for half, (wt, dsb) in enumerate(((ut, de_sb), (vt, do_sb))):
                for jc in range(2):
                    pt = ps1.tile([P, FC], F32, tag="acc", name="pt")
                    for rt in range(KT2):
                        nc.tensor.matmul(
                            pt[:],
                            lhsT=wt[:, rt * P : (rt + 1) * P],
                            rhs=dsb[rt][:, jc * FC : (jc + 1) * FC],
                            start=(rt == 0),
                            stop=(rt == KT2 - 1),
                        )
                    tt = t_pool.tile([P, FC], F32R, tag="t", name="tt")
                    nc.vector.tensor_copy(tt[:], pt[:])
                    col0 = half * H + jc * FC
                    # Bottom-half blocks (cb>=8) arrive partition-reversed
                    # (host reversed their lhsT columns) and are stored
                    # mirror-ordered: row 1024+k holds T[2047-k].  Then the
                    # pass-2 fold reads both halves with plain ascending loads.
                    row0 = cb * P if cb < KT2 else (23 * P - cb * P)
                    nc.scalar.dma_start(
                        t_dram[row0 : row0 + P, col0 : col0 + FC], tt[:]
                    )

        # ---- pass 2: fold T over rows, transform, de-interleave out ----
        # bot_rev[c', f] = T[2047-c', f]: partition reversal via one PE matmul
        # with the reversal permutation R (out[m,n] = bot[127-m, n]); the
        # tile-order flip (ct -> 7-ct) via a reversed free-dim view in the add.
        # Software-pipelined: loads run 3 blocks ahead, reversal matmul + DVE
        # fold 2 ahead, so block fb's chains never wait on its fold.
        folded: dict = {}

        def p2_load(fb):
            top = b_pool.tile([P, H], F32R, tag="top", name="top")
            nc.sync.dma_start(
                top[:].rearrange("p (t m) -> p t m", t=KT2),
                t_dram[0:H, fb * P : (fb + 1) * P].rearrange(
                    "(t p) m -> p t m", p=P
                ),
            )
            bot = b_pool.tile([P, H], F32R, tag="bot", name="bot")
            nc.sync.dma_start(
                bot[:].rearrange("p (t m) -> p t m", t=KT2),
                t_dram[H:N, fb * P : (fb + 1) * P].rearrange(
                    "(t p) m -> p t m", p=P
                ),
            )
            folded[fb] = (top, bot)

        def p2_fold(fb):
            # mirror-ordered bottom storage makes the fold a plain 2D add/sub
            top, bot = folded[fb]
            u2 = b_pool.tile([P, H], F32R, tag="u2", name="u2")
            nc.vector.tensor_add(u2[:], top[:], bot[:])
            v2 = b_pool.tile([P, H], F32R, tag="v2", name="v2")
            nc.vector.tensor_sub(v2[:], top[:], bot[:])
            folded[fb] = (u2, v2)

        p2_load(0)
        p2_load(1)
        p2_fold(0)
        p2_load(2)
        p2_fold(1)
        for fb in range(KT):
            u2, v2 = folded.pop(fb)
            # f_blk block fb -> actual Z rows (de-interleave rows via stride 2)
            if fb < KT2:
                row0 = 2 * fb * P
                row_stop = row0 + 2 * P
            else:
                row0 = 2 * (fb - KT2) * P + 1
                row_stop = row0 + 2 * P - 1
            for jc in range(2):
                pe_ = ps2.tile([P, FC], F32, tag="zacc", name="pe_")
                for ct in range(KT2):
                    nc.tensor.matmul(
                        pe_[:],
                        lhsT=u2[:, ct * P : (ct + 1) * P],
                        rhs=de_sb[ct][:, jc * FC : (jc + 1) * FC],
                        start=(ct == 0),
                        stop=(ct == KT2 - 1),
                    )
                po_ = ps2.tile([P, FC], F32, tag="zacc", name="po_")
                for ct in range(KT2):
                    nc.tensor.matmul(
                        po_[:],
                        lhsT=v2[:, ct * P : (ct + 1) * P],
                        rhs=do_sb[ct][:, jc * FC : (jc + 1) * FC],
                        start=(ct == 0),
                        stop=(ct == KT2 - 1),
                    )
                zt = z_pool.tile([P, 2 * FC], F32, tag="z", name="zt")
                nc.scalar.copy(zt[:, 0 : 2 * FC : 2], pe_[:])
                nc.vector.tensor_copy(zt[:, 1 : 2 * FC : 2], po_[:])
                nc.scalar.dma_start(
                    z_ext[row0:row_stop:2, jc * 2 * FC : (jc + 1) * 2 * FC],
                    zt[:],
                )
            if fb + 3 < KT:
                p2_load(fb + 3)
            if fb + 2 < KT:
                p2_fold(fb + 2)

    nc.finalize()
    return nc


def _build_bfly16() -> bass.Bass:
    """bf16 radix-2 butterfly DCT with the intermediate T kept entirely in
    SBUF (no DRAM roundtrip).  Same math/layout as _build_bfly: pass-1 folding
    host-side (u/v uploaded, bottom column-blocks mirrored), blocked T
    [even|odd], pass-2 fold via partition-aligned mirror blocks, outputs
    de-interleaved on chip + stride-2 row DMA stores.  bf16 operands double
    the PE rate vs fp32r and halve SBUF/DMA footprint; PSUM accumulates f32.
    """
    nc = bacc.Bacc(None, target_bir_lowering=False)
    u_ext = nc.declare_dram_parameter("u", [H, N], BF16, isOutput=False)
    v_ext = nc.declare_dram_parameter("v", [H, N], BF16, isOutput=False)
    de_ext = nc.declare_dram_parameter("de", [H, H], BF16, isOutput=False)
    do_ext = nc.declare_dram_parameter("do", [H, H], BF16, isOutput=False)
    z_ext = nc.declare_dram_parameter("z", [N, N], F32, isOutput=True)

    with ExitStack() as ctx:
        tc = ctx.enter_context(tile.TileContext(nc))
        d_pool = ctx.enter_context(tc.tile_pool(name="d", bufs=1))
        x_pool = ctx.enter_context(tc.tile_pool(name="x", bufs=4))
        t_pool = ctx.enter_context(tc.tile_pool(name="t", bufs=1))
        f_pool = ctx.enter_context(tc.tile_pool(name="f", bufs=1))
        z_pool = ctx.enter_context(tc.tile_pool(name="z", bufs=3))
        ps1 = ctx.enter_context(tc.tile_pool(name="ps1", bufs=2, space="PSUM"))
        ps2 = ctx.enter_context(tc.tile_pool(name="ps2", bufs=6, space="PSUM"))

        de_sb = [
            d_pool.tile([P, H], BF16, tag=f"de{t}", name=f"de{t}")
            for t in range(KT2)
        ]
        do_sb = [
            d_pool.tile([P, H], BF16, tag=f"do{t}", name=f"do{t}")
            for t in range(KT2)
        ]
        # T resident in SBUF: storage block s holds pass-1 output of
        # cb = s (s<8) or cb = 23-s (s>=8, partition-mirrored rows).
        t_sb = [
            t_pool.tile([P, N], BF16, tag=f"t{s}", name=f"t{s}")
            for s in range(KT)
        ]

        def load_block(ext, cb, tag):
            w = x_pool.tile([P, H], BF16, tag=tag, name="w_" + tag)
            nc.sync.dma_start(
                w[:].rearrange("p (t m) -> p t m", t=KT2),
                ext[:, cb * P : (cb + 1) * P].rearrange("(t p) m -> p t m", p=P),
            )
            return w

        u0 = load_block(u_ext, 0, "u")
        for jc in range(2):
            for t in range(KT2):
                nc.sync.dma_start(
                    de_sb[t][:, jc * FC : (jc + 1) * FC],
                    de_ext[t * P : (t + 1) * P, jc * FC : (jc + 1) * FC],
                )
        v0 = load_block(v_ext, 0, "v")
        for jc in range(2):
            for t in range(KT2):
                nc.sync.dma_start(
                    do_sb[t][:, jc * FC : (jc + 1) * FC],
                    do_ext[t * P : (t + 1) * P, jc * FC : (jc + 1) * FC],
                )

        # ---- pass 1: T_blk[cb, :] straight into SBUF ----
        for cb in range(KT):
            ut = u0 if cb == 0 else load_block(u_ext, cb, "u")
            vt = v0 if cb == 0 else load_block(v_ext, cb, "v")
            s = cb if cb < KT2 else 23 - cb
            for half, (wt, dsb) in enumerate(((ut, de_sb), (vt, do_sb))):
                for jc in range(2):
                    pt = ps1.tile([P, FC], F32, tag="acc", name="pt")
                    for rt in range(KT2):
                        nc.tensor.matmul(
                            pt[:],
                            lhsT=wt[:, rt * P : (rt + 1) * P],
                            rhs=dsb[rt][:, jc * FC : (jc + 1) * FC],
                            start=(rt == 0),
                            stop=(rt == KT2 - 1),
                        )
                    col0 = half * H + jc * FC
                    nc.vector.tensor_copy(
                        t_sb[s][:, col0 : col0 + FC], pt[:]
                    )

        # ---- pass 2: fold T in SBUF, transform, de-interleave out ----
        u2 = [
            f_pool.tile([P, N], BF16, tag=f"u2_{t}", name=f"u2_{t}")
            for t in range(KT2)
        ]
        v2 = [
            f_pool.tile([P, N], BF16, tag=f"v2_{t}", name=f"v2_{t}")
            for t in range(KT2)
        ]
        # fold order t=7..0: pair (t, 8+t) is ready once cb=15-t finished,
        # so later folds wait on earlier pass-1 chains.
        for t in range(KT2 - 1, -1, -1):
            nc.vector.tensor_add(u2[t][:], t_sb[t][:], t_sb[KT2 + t][:])
            nc.vector.tensor_sub(v2[t][:], t_sb[t][:], t_sb[KT2 + t][:])

        for fb in range(KT):
            if fb < KT2:
                row0 = 2 * fb * P
                row_stop = row0 + 2 * P
            else:
                row0 = 2 * (fb - KT2) * P + 1
                row_stop = row0 + 2 * P - 1
            for jc in range(2):
                pe_ = ps2.tile([P, FC], F32, tag="zacc", name="pe_")
                for ct in range(KT2 - 1, -1, -1):
                    nc.tensor.matmul(
                        pe_[:],
                        lhsT=u2[ct][:, fb * P : (fb + 1) * P],
                        rhs=de_sb[ct][:, jc * FC : (jc + 1) * FC],
                        start=(ct == KT2 - 1),
                        stop=(ct == 0),
                    )
                po_ = ps2.tile([P, FC], F32, tag="zacc", name="po_")
                for ct in range(KT2 - 1, -1, -1):
                    nc.tensor.matmul(
                        po_[:],
                        lhsT=v2[ct][:, fb * P : (fb + 1) * P],
                        rhs=do_sb[ct][:, jc * FC : (jc + 1) * FC],
                        start=(ct == KT2 - 1),
                        stop=(ct == 0),
                    )
                zt = z_pool.tile([P, 2 * FC], F32, tag="z", name="zt")
                nc.scalar.copy(zt[:, 0 : 2 * FC : 2], pe_[:])
                nc.vector.tensor_copy(zt[:, 1 : 2 * FC : 2], po_[:])
                nc.scalar.dma_start(
                    z_ext[row0:row_stop:2, jc * 2 * FC : (jc + 1) * 2 * FC],
                    zt[:],
                )

    nc.finalize()
    return nc


def _build_split() -> bass.Bass:
    """hi/lo bf16 decomposition: each logical matmul = 3 bf16 matmuls
    (Xh Dh + Xh Dl + Xl Dh), accumulated in the same PSUM chain."""
    nc = bacc.Bacc(None, target_bir_lowering=False)
    xh_ext = nc.declare_dram_parameter("xh", [N, N], BF16, isOutput=False)
    xl_ext = nc.declare_dram_parameter("xl", [N, N], BF16, isOutput=False)
    dh_ext = nc.declare_dram_parameter("dh", [N, N], BF16, isOutput=False)
    dl_ext = nc.declare_dram_parameter("dl", [N, N], BF16, isOutput=False)
    z_ext = nc.declare_dram_parameter("z", [N, N], F32, isOutput=True)

    with ExitStack() as ctx:
        tc = ctx.enter_context(tile.TileContext(nc))
        d_pool = ctx.enter_context(tc.tile_pool(name="d", bufs=1))
        x_pool = ctx.enter_context(tc.tile_pool(name="x", bufs=3))
        w_pool = ctx.enter_context(tc.tile_pool(name="w", bufs=3))
        t_pool = ctx.enter_context(tc.tile_pool(name="t", bufs=KT))
        z_pool = ctx.enter_context(tc.tile_pool(name="z", bufs=3))
        ps1 = ctx.enter_context(tc.tile_pool(name="ps1", bufs=4, space="PSUM"))
        ps2 = ctx.enter_context(tc.tile_pool(name="ps2", bufs=4, space="PSUM"))

        dh_sb = [
            d_pool.tile([P, N], BF16, tag=f"dh{t}", name=f"dh{t}")
            for t in range(KT)
        ]
        dl_sb = [
            d_pool.tile([P, N], BF16, tag=f"dl{t}", name=f"dl{t}")
            for t in range(KT)
        ]
        for fcol in range(NFC):
            for t in range(KT):
                nc.sync.dma_start(
                    dh_sb[t][:, fcol * FC : (fcol + 1) * FC],
                    dh_ext[t * P : (t + 1) * P, fcol * FC : (fcol + 1) * FC],
                )
                nc.sync.dma_start(
                    dl_sb[t][:, fcol * FC : (fcol + 1) * FC],
                    dl_ext[t * P : (t + 1) * P, fcol * FC : (fcol + 1) * FC],
                )

        for fc in range(NFC):
            t_tiles = []
            for cb in range(KT):
                xht = x_pool.tile([P, N], BF16, tag="xh", name="xht")
                xlt = x_pool.tile([P, N], BF16, tag="xl", name="xlt")
                for t_, ext in ((xht, xh_ext), (xlt, xl_ext)):
                    nc.sync.dma_start(
                        t_[:].rearrange("p (t m) -> p t m", t=KT),
                        ext[:, cb * P : (cb + 1) * P].rearrange(
                            "(t p) m -> p t m", p=P
                        ),
                    )
                pt = ps1.tile([P, FC], F32, tag="ps1", name="pt")
                nmm = 3 * KT
                i = 0
                for rt in range(KT):
                    dh = dh_sb[rt][:, fc * FC : (fc + 1) * FC]
                    dl = dl_sb[rt][:, fc * FC : (fc + 1) * FC]
                    xh = xht[:, rt * P : (rt + 1) * P]
                    xl = xlt[:, rt * P : (rt + 1) * P]
                    for l_, r_ in ((xh, dh), (xh, dl), (xl, dh)):
                        nc.tensor.matmul(
                            pt[:], lhsT=l_, rhs=r_,
                            start=(i == 0), stop=(i == nmm - 1),
                        )
                        i += 1
                # split T on device: th = bf16(T), tl = bf16(T - th)
                th = t_pool.tile([P, FC], BF16, tag="th", name="th")
                tl = t_pool.tile([P, FC], BF16, tag="tl", name="tl")
                tmp = w_pool.tile([P, FC], F32, tag="tmp", name="tmp")
                nc.vector.tensor_copy(th[:], pt[:])
                nc.scalar.copy(tmp[:], th[:])
                nc.vector.tensor_sub(tmp[:], pt[:], tmp[:])
                nc.vector.tensor_copy(tl[:], tmp[:])
                t_tiles.append((th, tl))

            for fb in range(FC // P):
                for g in range(NFC):
                    pz = ps2.tile([P, FC], F32, tag="ps2", name="pz")
                    nmm = 3 * KT
                    i = 0
                    for ct in range(KT):
                        th, tl = t_tiles[ct]
                        dh = dh_sb[ct][:, g * FC : (g + 1) * FC]
                        dl = dl_sb[ct][:, g * FC : (g + 1) * FC]
                        thb = th[:, fb * P : (fb + 1) * P]
                        tlb = tl[:, fb * P : (fb + 1) * P]
                        for l_, r_ in ((thb, dh), (thb, dl), (tlb, dh)):
                            nc.tensor.matmul(
                                pz[:], lhsT=l_, rhs=r_,
                                start=(i == 0), stop=(i == nmm - 1),
                            )
                            i += 1
                    zt = z_pool.tile([P, FC], F32, tag="z", name="zt")
                    nc.vector.tensor_copy(zt[:], pz[:])
                    row0 = (fc * (FC // P) + fb) * P
                    nc.sync.dma_start(
                        z_ext[row0 : row0 + P, g * FC : (g + 1) * FC], zt[:]
                    )

    nc.finalize()
    return nc


_PROGRAM_CACHE: dict = {}


_BUILDERS = {
    "f32r": _build_f32r,
    "bfly": _build_bfly,
    "bfly16": _build_bfly16,
    "split": _build_split,
}


def _get_program(mode: str) -> bass.Bass:
    if mode not in _PROGRAM_CACHE:
        _PROGRAM_CACHE[mode] = _BUILDERS[mode]()
    return _PROGRAM_CACHE[mode]


def _make_in_maps(x: np.ndarray, mode: str):
    if mode == "f32r":
        dr = _round_f32r(_dct_matrix_d())
        return [{"x": _round_f32r(x[i]), "d": dr} for i in range(B)]
    if mode == "bfly16":
        import ml_dtypes

        n2 = np.arange(H, dtype=np.float64)[:, None]
        j2 = np.arange(H, dtype=np.float64)[None, :]
        de = np.cos(np.pi * (2 * n2 + 1) * j2 / N).astype(ml_dtypes.bfloat16)
        do = np.cos(np.pi * (2 * n2 + 1) * (2 * j2 + 1) / (2 * N)).astype(
            ml_dtypes.bfloat16
        )
        maps = []
        for i in range(B):
            xi = np.asarray(x[i], dtype=np.float32)
            xr = xi[::-1]
            u = (xi[:H] + xr[:H]).astype(ml_dtypes.bfloat16)
            v = (xi[:H] - xr[:H]).astype(ml_dtypes.bfloat16)
            # Column-reverse blocks 8..15 so pass-1 bottom-half outputs come
            # out partition-mirrored (see _build_bfly16 pass-2 fold).
            for w in (u, v):
                blk = w[:, H:].reshape(H, KT2, P)
                w[:, H:] = blk[:, :, ::-1].reshape(H, H)
            maps.append({"u": u, "v": v, "de": de, "do": do})
        return maps
    if mode == "bfly":
        n2 = np.arange(H, dtype=np.float64)[:, None]
        j2 = np.arange(H, dtype=np.float64)[None, :]
        de = _round_f32r(np.cos(np.pi * (2 * n2 + 1) * j2 / N).astype(np.float32))
        do = _round_f32r(
            np.cos(np.pi * (2 * n2 + 1) * (2 * j2 + 1) / (2 * N)).astype(
                np.float32
            )
        )
        maps = []
        for i in range(B):
            xi = np.asarray(x[i], dtype=np.float32)
            xr = xi[::-1]
            u = _round_f32r(xi[:H] + xr[:H])
            v = _round_f32r(xi[:H] - xr[:H])
            # Column-reverse blocks 8..15: pass-1 output partitions for those
            # blocks then come out mirror-ordered, which makes the pass-2
            # bottom-half fold a plain ascending read (see _build_bfly).
            for w in (u, v):
                blk = w[:, H:].reshape(H, KT2, P)
                w[:, H:] = blk[:, :, ::-1].reshape(H, H)
            maps.append({"u": u, "v": v, "de": de, "do": do})
        return maps
    dh, dl = _split_bf16(_dct_matrix_d())
    maps = []
    for i in range(B):
        xh, xl = _split_bf16(np.ascontiguousarray(x[i], dtype=np.float32))
        maps.append({"xh": xh, "xl": xl, "dh": dh, "dl": dl})
    return maps


def kernel(x: np.ndarray) -> np.ndarray:
    x = np.asarray(x)
    assert x.shape == (B, N, N), x.shape
    nc = _get_program(MODE)
    in_maps = _make_in_maps(x, MODE)
    res = run_bass_kernel_spmd(nc, in_maps, list(range(B)))
    out = np.stack([res.results[i]["z"] for i in range(B)], axis=0)
    return out.astype(np.float32, copy=False)



# revision 17
# speedup vs baseline: 1.2311x; 1.0517x over previous
"""Trainium2 Bass kernel: batched 2D DCT-II (unnormalized), x: (8, 2048, 2048) f32.

Math: per image X, the unnormalized 2D DCT-II is Z = C @ X @ C^T with
C[k,n] = cos(pi*(2n+1)*k/(2N)).  Let D = C^T.  Using the PE's
out = lhsT.T @ rhs semantics with the *data* as the stationary operand and D as
the moving operand, the two passes chain with no transposes:
    pass 1:  T = X^T @ D      (lhsT = X tiles,  rhs = D)   -> T[c, f]
    pass 2:  Z = T^T @ D      (lhsT = T tiles,  rhs = D)   -> Z = D^T X D = C X C^T

Sharding: batch dim 8 -> one image per NeuronCore (data parallel, no comms).

Dtype modes:
  "f32r"  - single fp32r matmul per term (full PE rate; TF32-like 11-bit
            mantissa operand rounding; ~2e-4 relative-to-absmax error).
  "split" - hi/lo bf16 decomposition, 3 matmuls per term (~5e-6 error, 3x cost).
"""

import numpy as np
from contextlib import ExitStack

import concourse.bass as bass
import concourse.bacc as bacc
import concourse.tile as tile
from concourse import mybir
from concourse.bass_utils import run_bass_kernel_spmd

F32 = mybir.dt.float32
F32R = mybir.dt.float32r
BF16 = mybir.dt.bfloat16

import os

MODE = os.environ.get("DCT_MODE", "lvl2")  # "lvl2", "bfly16", "bfly", "f32r", "split"

B = 8          # batch == n_cores
N = 2048       # image is N x N
P = 128        # partitions
KT = N // P    # 16 k-tiles along any contraction
FC = 512       # chunk width (pass-1 f-chunk, pass-2 g-chunk, PSUM bank)
NFC = N // FC  # 4 chunks
H = N // 2     # butterfly half size
KT2 = H // P   # 8 k-tiles at half contraction


def _round_f32r(a: np.ndarray) -> np.ndarray:
    """fp32r = round-to-nearest, 11 explicit mantissa bits (drop low 12)."""
    b = np.ascontiguousarray(a, dtype=np.float32).view(np.uint32)
    r = ((b + np.uint32(0x800)) & np.uint32(0xFFFFF000)).view(np.float32)
    return r


def _split_bf16(a: np.ndarray):
    import ml_dtypes

    hi = a.astype(ml_dtypes.bfloat16)
    lo = (a - hi.astype(np.float32)).astype(ml_dtypes.bfloat16)
    return hi, lo


def _dct_matrix_d() -> np.ndarray:
    # D[n, k] = cos(pi * (2n+1) * k / (2N)), exact in float64
    n = np.arange(N, dtype=np.float64)[:, None]
    k = np.arange(N, dtype=np.float64)[None, :]
    d = np.cos(np.pi * (2.0 * n + 1.0) * k / (2.0 * N))
    return d.astype(np.float32)


def _build_f32r() -> bass.Bass:
    """fp32r two-pass DCT with the intermediate T round-tripped via DRAM.

    Pass 1 streams X once (one column-block per chain, all 4 f-chunks while
    the block is resident).  T chunks are written back to a DRAM scratch and
    re-streamed as pass-2 stationary tiles.  D stays resident in SBUF.
    """
    nc = bacc.Bacc(None, target_bir_lowering=False)
    x_ext = nc.declare_dram_parameter("x", [N, N], F32R, isOutput=False)
    d_ext = nc.declare_dram_parameter("d", [N, N], F32R, isOutput=False)
    z_ext = nc.declare_dram_parameter("z", [N, N], F32, isOutput=True)

    with ExitStack() as ctx:
        tc = ctx.enter_context(tile.TileContext(nc))
        d_pool = ctx.enter_context(tc.tile_pool(name="d", bufs=1))
        x_pool = ctx.enter_context(tc.tile_pool(name="x", bufs=3))
        t_pool = ctx.enter_context(tc.tile_pool(name="t", bufs=6))
        z_pool = ctx.enter_context(tc.tile_pool(name="z", bufs=3))
        dram = ctx.enter_context(tc.tile_pool(name="dram", bufs=1, space="DRAM"))
        ps1 = ctx.enter_context(tc.tile_pool(name="ps1", bufs=4, space="PSUM"))
        ps2 = ctx.enter_context(tc.tile_pool(name="ps2", bufs=4, space="PSUM"))

        t_dram = dram.tile([N, N], F32R, name="t_dram")

        # First column-block of X loads before D so pass 1 starts early.
        d_sb = [
            d_pool.tile([P, N], F32R, tag=f"d{t}", name=f"d{t}") for t in range(KT)
        ]

        def load_x(cb):
            xt = x_pool.tile([P, N], F32R, tag="x", name="xt")
            nc.sync.dma_start(
                xt[:].rearrange("p (t m) -> p t m", t=KT),
                x_ext[:, cb * P : (cb + 1) * P].rearrange("(t p) m -> p t m", p=P),
            )
            return xt

        x0 = load_x(0)
        # D f-chunk 0 for all 16 row-tiles (pass-1 chain 0 needs only these)
        for fcol in range(NFC):
            for t in range(KT):
                nc.sync.dma_start(
                    d_sb[t][:, fcol * FC : (fcol + 1) * FC],
                    d_ext[t * P : (t + 1) * P, fcol * FC : (fcol + 1) * FC],
                )
            if fcol == 0:
                # remaining D chunks stream behind pass-1 compute
                pass

        # ---- pass 1: per column-block cb, all f-chunks: T[cb,:] = (X^T D)[cb,:]
        for cb in range(KT):
            xt = x0 if cb == 0 else load_x(cb)
            for fc in range(NFC):
                pt = ps1.tile([P, FC], F32, tag="ps1", name="pt")
                for rt in range(KT):
                    nc.tensor.matmul(
                        pt[:],
                        lhsT=xt[:, rt * P : (rt + 1) * P],
                        rhs=d_sb[rt][:, fc * FC : (fc + 1) * FC],
                        start=(rt == 0),
                        stop=(rt == KT - 1),
                    )
                tt = t_pool.tile([P, FC], F32R, tag="t", name="tt")
                nc.vector.tensor_copy(tt[:], pt[:])
                nc.scalar.dma_start(
                    t_dram[cb * P : (cb + 1) * P, fc * FC : (fc + 1) * FC], tt[:]
                )

        # ---- pass 2: per f-block fb: Z[fb,:] = (T^T D)[fb,:]
        for fb in range(KT):
            tf = x_pool.tile([P, N], F32R, tag="x", name="tf")
            nc.sync.dma_start(
                tf[:].rearrange("p (t m) -> p t m", t=KT),
                t_dram[:, fb * P : (fb + 1) * P].rearrange("(t p) m -> p t m", p=P),
            )
            for g in range(NFC):
                pz = ps2.tile([P, FC], F32, tag="ps2", name="pz")
                for ct in range(KT):
                    nc.tensor.matmul(
                        pz[:],
                        lhsT=tf[:, ct * P : (ct + 1) * P],
                        rhs=d_sb[ct][:, g * FC : (g + 1) * FC],
                        start=(ct == 0),
                        stop=(ct == KT - 1),
                    )
                zt = z_pool.tile([P, FC], F32, tag="z", name="zt")
                nc.vector.tensor_copy(zt[:], pz[:])
                nc.scalar.dma_start(
                    z_ext[fb * P : (fb + 1) * P, g * FC : (g + 1) * FC], zt[:]
                )

    nc.finalize()
    return nc


def _build_bfly() -> bass.Bass:
    """Radix-2 even/odd DCT factorization in fp32r: each 1D DCT-II of size N
    becomes two size-N/2 cosine transforms of the folded sequences
    u = x_top + reverse(x_bot), v = x_top - reverse(x_bot):
        y[2j]   = sum_{n<H} u[n] De[n, j],   De[n,j] = cos(pi (2n+1) j / N)
        y[2j+1] = sum_{n<H} v[n] Do[n, j],   Do[n,j] = cos(pi (2n+1)(2j+1) / 2N)
    Halves the matmul work per pass.  Pass-1 folding is done on the host
    (u/v uploaded); pass-2 folding of the intermediate T is done by DVE with a
    reversed-row DMA load.  Outputs are de-interleaved on chip (strided DVE
    writes) + stride-2-row DMA stores, so all DRAM traffic stays contiguous
    per partition.
    """
    nc = bacc.Bacc(None, target_bir_lowering=False)
    u_ext = nc.declare_dram_parameter("u", [H, N], F32R, isOutput=False)
    v_ext = nc.declare_dram_parameter("v", [H, N], F32R, isOutput=False)
    de_ext = nc.declare_dram_parameter("de", [H, H], F32R, isOutput=False)
    do_ext = nc.declare_dram_parameter("do", [H, H], F32R, isOutput=False)
    z_ext = nc.declare_dram_parameter("z", [N, N], F32, isOutput=True)

    with ExitStack() as ctx:
        tc = ctx.enter_context(tile.TileContext(nc))
        d_pool = ctx.enter_context(tc.tile_pool(name="d", bufs=1))
        x_pool = ctx.enter_context(tc.tile_pool(name="x", bufs=4))
        t_pool = ctx.enter_context(tc.tile_pool(name="t", bufs=4))
        b_pool = ctx.enter_context(tc.tile_pool(name="b", bufs=4))
        z_pool = ctx.enter_context(tc.tile_pool(name="z", bufs=3))
        dram = ctx.enter_context(tc.tile_pool(name="dram", bufs=1, space="DRAM"))
        # PSUM: pass-1 accumulators 2x1 bank; pass-2 output chains share one
        # 6-slot pool (6 banks) so slow de-interleave copies never stall the
        # next chain. 8 banks total (no reversal matmuls anymore).
        ps1 = ctx.enter_context(tc.tile_pool(name="ps1", bufs=2, space="PSUM"))
        ps2 = ctx.enter_context(tc.tile_pool(name="ps2", bufs=6, space="PSUM"))

        # T in blocked layout: cols [0,H) = even outputs, [H,2H) = odd
        t_dram = dram.tile([N, N], F32R, name="t_dram")

        de_sb = [
            d_pool.tile([P, H], F32R, tag=f"de{t}", name=f"de{t}")
            for t in range(KT2)
        ]
        do_sb = [
            d_pool.tile([P, H], F32R, tag=f"do{t}", name=f"do{t}")
            for t in range(KT2)
        ]

        def load_block(ext, cb, tag):
            w = x_pool.tile([P, H], F32R, tag=tag, name="w_" + tag)
            nc.sync.dma_start(
                w[:].rearrange("p (t m) -> p t m", t=KT2),
                ext[:, cb * P : (cb + 1) * P].rearrange("(t p) m -> p t m", p=P),
            )
            return w

        # loads in exact first-consumption order: u0, de jc0, de jc1, v0,
        # do jc0, do jc1; the pass-2 reversal matrix r last.
        u0 = load_block(u_ext, 0, "u")
        for jc in range(2):
            for t in range(KT2):
                nc.sync.dma_start(
                    de_sb[t][:, jc * FC : (jc + 1) * FC],
                    de_ext[t * P : (t + 1) * P, jc * FC : (jc + 1) * FC],
                )
        v0 = load_block(v_ext, 0, "v")
        for jc in range(2):
            for t in range(KT2):
                nc.sync.dma_start(
                    do_sb[t][:, jc * FC : (jc + 1) * FC],
                    do_ext[t * P : (t + 1) * P, jc * FC : (jc + 1) * FC],
                )

        # ---- pass 1: T_blk[cb, :] ----
        for cb in range(KT):
            ut = u0 if cb == 0 else load_block(u_ext, cb, "u")
            vt = v0 if cb == 0 else load_block(v_ext, cb, "v")
            for half, (wt, dsb) in enumerate(((ut, de_sb), (vt, do_sb))):
                for jc in range(2):
                    pt = ps1.tile([P, FC], F32, tag="acc", name="pt")
                    for rt in range(KT2):
                        nc.tensor.matmul(
                            pt[:],
                            lhsT=wt[:, rt * P : (rt + 1) * P],
                            rhs=dsb[rt][:, jc * FC : (jc + 1) * FC],
                            start=(rt == 0),
                            stop=(rt == KT2 - 1),
                        )
                    tt = t_pool.tile([P, FC], F32R, tag="t", name="tt")
                    nc.vector.tensor_copy(tt[:], pt[:])
                    col0 = half * H + jc * FC
                    # Bottom-half blocks (cb>=8) arrive partition-reversed
                    # (host reversed their lhsT columns) and are stored
                    # mirror-ordered: row 1024+k holds T[2047-k].  Then the
                    # pass-2 fold reads both halves with plain ascending loads.
                    row0 = cb * P if cb < KT2 else (23 * P - cb * P)
                    nc.scalar.dma_start(
                        t_dram[row0 : row0 + P, col0 : col0 + FC], tt[:]
                    )

        # ---- pass 2: fold T over rows, transform, de-interleave out ----
        # bot_rev[c', f] = T[2047-c', f]: partition reversal via one PE matmul
        # with the reversal permutation R (out[m,n] = bot[127-m, n]); the
        # tile-order flip (ct -> 7-ct) via a reversed free-dim view in the add.
        # Software-pipelined: loads run 3 blocks ahead, reversal matmul + DVE
        # fold 2 ahead, so block fb's chains never wait on its fold.
        folded: dict = {}

        def p2_load(fb):
            top = b_pool.tile([P, H], F32R, tag="top", name="top")
            nc.sync.dma_start(
                top[:].rearrange("p (t m) -> p t m", t=KT2),
                t_dram[0:H, fb * P : (fb + 1) * P].rearrange(
                    "(t p) m -> p t m", p=P
                ),
            )
            bot = b_pool.tile([P, H], F32R, tag="bot", name="bot")
            nc.sync.dma_start(
                bot[:].rearrange("p (t m) -> p t m", t=KT2),
                t_dram[H:N, fb * P : (fb + 1) * P].rearrange(
                    "(t p) m -> p t m", p=P
                ),
            )
            folded[fb] = (top, bot)

        def p2_fold(fb):
            # mirror-ordered bottom storage makes the fold a plain 2D add/sub
            top, bot = folded[fb]
            u2 = b_pool.tile([P, H], F32R, tag="u2", name="u2")
            nc.vector.tensor_add(u2[:], top[:], bot[:])
            v2 = b_pool.tile([P, H], F32R, tag="v2", name="v2")
            nc.vector.tensor_sub(v2[:], top[:], bot[:])
            folded[fb] = (u2, v2)

        p2_load(0)
        p2_load(1)
        p2_fold(0)
        p2_load(2)
        p2_fold(1)
        for fb in range(KT):
            u2, v2 = folded.pop(fb)
            # f_blk block fb -> actual Z rows (de-interleave rows via stride 2)
            if fb < KT2:
                row0 = 2 * fb * P
                row_stop = row0 + 2 * P
            else:
                row0 = 2 * (fb - KT2) * P + 1
                row_stop = row0 + 2 * P - 1
            for jc in range(2):
                pe_ = ps2.tile([P, FC], F32, tag="zacc", name="pe_")
                for ct in range(KT2):
                    nc.tensor.matmul(
                        pe_[:],
                        lhsT=u2[:, ct * P : (ct + 1) * P],
                        rhs=de_sb[ct][:, jc * FC : (jc + 1) * FC],
                        start=(ct == 0),
                        stop=(ct == KT2 - 1),
                    )
                po_ = ps2.tile([P, FC], F32, tag="zacc", name="po_")
                for ct in range(KT2):
                    nc.tensor.matmul(
                        po_[:],
                        lhsT=v2[:, ct * P : (ct + 1) * P],
                        rhs=do_sb[ct][:, jc * FC : (jc + 1) * FC],
                        start=(ct == 0),
                        stop=(ct == KT2 - 1),
                    )
                zt = z_pool.tile([P, 2 * FC], F32, tag="z", name="zt")
                nc.scalar.copy(zt[:, 0 : 2 * FC : 2], pe_[:])
                nc.vector.tensor_copy(zt[:, 1 : 2 * FC : 2], po_[:])
                nc.scalar.dma_start(
                    z_ext[row0:row_stop:2, jc * 2 * FC : (jc + 1) * 2 * FC],
                    zt[:],
                )
            if fb + 3 < KT:
                p2_load(fb + 3)
            if fb + 2 < KT:
                p2_fold(fb + 2)

    nc.finalize()
    return nc


def _build_bfly16() -> bass.Bass:
    """bf16 radix-2 butterfly DCT with the intermediate T kept entirely in
    SBUF (no DRAM roundtrip).  Same math/layout as _build_bfly: pass-1 folding
    host-side (u/v uploaded, bottom column-blocks mirrored), blocked T
    [even|odd], pass-2 fold via partition-aligned mirror blocks, outputs
    de-interleaved on chip + stride-2 row DMA stores.  bf16 operands double
    the PE rate vs fp32r and halve SBUF/DMA footprint; PSUM accumulates f32.
    """
    nc = bacc.Bacc(None, target_bir_lowering=False)
    u_ext = nc.declare_dram_parameter("u", [H, N], BF16, isOutput=False)
    v_ext = nc.declare_dram_parameter("v", [H, N], BF16, isOutput=False)
    de_ext = nc.declare_dram_parameter("de", [H, H], BF16, isOutput=False)
    do_ext = nc.declare_dram_parameter("do", [H, H], BF16, isOutput=False)
    z_ext = nc.declare_dram_parameter("z", [N, N], F32, isOutput=True)

    with ExitStack() as ctx:
        tc = ctx.enter_context(tile.TileContext(nc))
        d_pool = ctx.enter_context(tc.tile_pool(name="d", bufs=1))
        x_pool = ctx.enter_context(tc.tile_pool(name="x", bufs=4))
        t_pool = ctx.enter_context(tc.tile_pool(name="t", bufs=1))
        f_pool = ctx.enter_context(tc.tile_pool(name="f", bufs=1))
        z_pool = ctx.enter_context(tc.tile_pool(name="z", bufs=3))
        ps1 = ctx.enter_context(tc.tile_pool(name="ps1", bufs=2, space="PSUM"))
        ps2 = ctx.enter_context(tc.tile_pool(name="ps2", bufs=6, space="PSUM"))

        de_sb = [
            d_pool.tile([P, H], BF16, tag=f"de{t}", name=f"de{t}")
            for t in range(KT2)
        ]
        do_sb = [
            d_pool.tile([P, H], BF16, tag=f"do{t}", name=f"do{t}")
            for t in range(KT2)
        ]
        # T resident in SBUF: storage block s holds pass-1 output of
        # cb = s (s<8) or cb = 23-s (s>=8, partition-mirrored rows).
        t_sb = [
            t_pool.tile([P, N], BF16, tag=f"t{s}", name=f"t{s}")
            for s in range(KT)
        ]

        def load_block(ext, cb, tag):
            w = x_pool.tile([P, H], BF16, tag=tag, name="w_" + tag)
            nc.sync.dma_start(
                w[:].rearrange("p (t m) -> p t m", t=KT2),
                ext[:, cb * P : (cb + 1) * P].rearrange("(t p) m -> p t m", p=P),
            )
            return w

        u0 = load_block(u_ext, 0, "u")
        for jc in range(2):
            for t in range(KT2):
                nc.sync.dma_start(
                    de_sb[t][:, jc * FC : (jc + 1) * FC],
                    de_ext[t * P : (t + 1) * P, jc * FC : (jc + 1) * FC],
                )
        v0 = load_block(v_ext, 0, "v")
        for jc in range(2):
            for t in range(KT2):
                nc.sync.dma_start(
                    do_sb[t][:, jc * FC : (jc + 1) * FC],
                    do_ext[t * P : (t + 1) * P, jc * FC : (jc + 1) * FC],
                )

        # ---- pass 1: T_blk[cb, :] straight into SBUF ----
        for cb in range(KT):
            ut = u0 if cb == 0 else load_block(u_ext, cb, "u")
            vt = v0 if cb == 0 else load_block(v_ext, cb, "v")
            s = cb if cb < KT2 else 23 - cb
            for half, (wt, dsb) in enumerate(((ut, de_sb), (vt, do_sb))):
                for jc in range(2):
                    pt = ps1.tile([P, FC], F32, tag="acc", name="pt")
                    for rt in range(KT2):
                        nc.tensor.matmul(
                            pt[:],
                            lhsT=wt[:, rt * P : (rt + 1) * P],
                            rhs=dsb[rt][:, jc * FC : (jc + 1) * FC],
                            start=(rt == 0),
                            stop=(rt == KT2 - 1),
                        )
                    col0 = half * H + jc * FC
                    nc.vector.tensor_copy(
                        t_sb[s][:, col0 : col0 + FC], pt[:]
                    )

        # ---- pass 2: fold T in SBUF, transform, de-interleave out ----
        u2 = [
            f_pool.tile([P, N], BF16, tag=f"u2_{t}", name=f"u2_{t}")
            for t in range(KT2)
        ]
        v2 = [
            f_pool.tile([P, N], BF16, tag=f"v2_{t}", name=f"v2_{t}")
            for t in range(KT2)
        ]
        # fold order t=7..0: pair (t, 8+t) is ready once cb=15-t finished,
        # so later folds wait on earlier pass-1 chains.
        for t in range(KT2 - 1, -1, -1):
            nc.vector.tensor_add(u2[t][:], t_sb[t][:], t_sb[KT2 + t][:])
            nc.vector.tensor_sub(v2[t][:], t_sb[t][:], t_sb[KT2 + t][:])

        for fb in range(KT):
            if fb < KT2:
                row0 = 2 * fb * P
                row_stop = row0 + 2 * P
            else:
                row0 = 2 * (fb - KT2) * P + 1
                row_stop = row0 + 2 * P - 1
            for jc in range(2):
                pe_ = ps2.tile([P, FC], F32, tag="zacc", name="pe_")
                for ct in range(KT2 - 1, -1, -1):
                    nc.tensor.matmul(
                        pe_[:],
                        lhsT=u2[ct][:, fb * P : (fb + 1) * P],
                        rhs=de_sb[ct][:, jc * FC : (jc + 1) * FC],
                        start=(ct == KT2 - 1),
                        stop=(ct == 0),
                    )
                po_ = ps2.tile([P, FC], F32, tag="zacc", name="po_")
                for ct in range(KT2 - 1, -1, -1):
                    nc.tensor.matmul(
                        po_[:],
                        lhsT=v2[ct][:, fb * P : (fb + 1) * P],
                        rhs=do_sb[ct][:, jc * FC : (jc + 1) * FC],
                        start=(ct == KT2 - 1),
                        stop=(ct == 0),
                    )
                zt = z_pool.tile([P, 2 * FC], F32, tag="z", name="zt")
                nc.scalar.copy(zt[:, 0 : 2 * FC : 2], pe_[:])
                nc.vector.tensor_copy(zt[:, 1 : 2 * FC : 2], po_[:])
                nc.scalar.dma_start(
                    z_ext[row0:row_stop:2, jc * 2 * FC : (jc + 1) * 2 * FC],
                    zt[:],
                )

    nc.finalize()
    return nc


def _build_lvl2() -> bass.Bass:
    """Level-2 pole-free butterfly DCT (bf16, T in SBUF).

    1D DCT-II_2048 factored twice:
      fold1: u = xt + xb_rev (DCT-II_1024), v = xt - xb_rev (DCT-IV_1024)
      u: fold2 -> uu (DCT-II_512), uv (DCT-IV_512)
      v (DCT-IV_1024, stable rotation form): av/bv Givens-rotated pairs,
         both through DCT-II_512 (bv with column-reversed matrix), then a
         post-butterfly of adjacent outputs: yodd[2j] = C[j] + S[j],
         yodd[2j+1] = C[j+1] - S[j+1], S[j] = SBV[Q-j].
    Pass-1 folds/rotations on host; pass-2 folds/rotations on device from
    SBUF-resident blocked T.  mu column permutation makes both pass-2 fold
    levels partition-aligned.  PE work is N^3/4 per pass (half of level-1).
    """
    nc = bacc.Bacc(None, target_bir_lowering=False)
    Qm = FC  # 512
    uu_ext = nc.declare_dram_parameter("uu", [Qm, N], BF16, isOutput=False)
    uv_ext = nc.declare_dram_parameter("uv", [Qm, N], BF16, isOutput=False)
    av_ext = nc.declare_dram_parameter("av", [Qm, N], BF16, isOutput=False)
    bv_ext = nc.declare_dram_parameter("bv", [Qm, N], BF16, isOutput=False)
    d2_ext = nc.declare_dram_parameter("d2", [Qm, Qm], BF16, isOutput=False)
    d4_ext = nc.declare_dram_parameter("d4", [Qm, Qm], BF16, isOutput=False)
    d2r_ext = nc.declare_dram_parameter("d2r", [Qm, Qm], BF16, isOutput=False)
    rot_ext = nc.declare_dram_parameter("rot", [P, 16], F32, isOutput=False)
    z_ext = nc.declare_dram_parameter("z", [N, N], F32, isOutput=True)

    AOT = __import__("concourse.alu_op_type", fromlist=["AluOpType"]).AluOpType

    with ExitStack() as ctx:
        tc = ctx.enter_context(tile.TileContext(nc))
        d_pool = ctx.enter_context(tc.tile_pool(name="d", bufs=1))
        x_pool = ctx.enter_context(tc.tile_pool(name="x", bufs=3))
        t_pool = ctx.enter_context(tc.tile_pool(name="t", bufs=1))
        f_pool = ctx.enter_context(tc.tile_pool(name="f", bufs=1))
        s_pool = ctx.enter_context(tc.tile_pool(name="s", bufs=1))
        z_pool = ctx.enter_context(tc.tile_pool(name="z", bufs=2))
        ps = ctx.enter_context(tc.tile_pool(name="ps", bufs=2, space="PSUM"))

        d2_sb = [d_pool.tile([P, Qm], BF16, tag=f"d2_{k}", name=f"d2_{k}") for k in range(4)]
        d4_sb = [d_pool.tile([P, Qm], BF16, tag=f"d4_{k}", name=f"d4_{k}") for k in range(4)]
        d2r_sb = [d_pool.tile([P, Qm], BF16, tag=f"d2r_{k}", name=f"d2r_{k}") for k in range(4)]
        rot_sb = d_pool.tile([P, 16], F32, tag="rot", name="rot")
        t_sb = [t_pool.tile([P, N], BF16, tag=f"t{s}", name=f"t{s}") for s in range(KT)]
        uu2 = [f_pool.tile([P, N], BF16, tag=f"uu2_{t}", name=f"uu2_{t}") for t in range(4)]
        uv2 = [f_pool.tile([P, N], BF16, tag=f"uv2_{t}", name=f"uv2_{t}") for t in range(4)]
        av2 = [f_pool.tile([P, N], BF16, tag=f"av2_{t}", name=f"av2_{t}") for t in range(4)]
        bv2 = [f_pool.tile([P, N], BF16, tag=f"bv2_{t}", name=f"bv2_{t}") for t in range(4)]

        def load_w(ext, cb, tag):
            w = x_pool.tile([P, 4 * P], BF16, tag=tag, name="w_" + tag)
            nc.sync.dma_start(
                w[:].rearrange("p (t m) -> p t m", t=4),
                ext[:, cb * P : (cb + 1) * P].rearrange("(t p) m -> p t m", p=P),
            )
            return w

        def load_d(ext, sb):
            for k in range(4):
                nc.sync.dma_start(sb[k][:], ext[k * P : (k + 1) * P, :])

        w0 = [
            load_w(uu_ext, 0, "uu"),
            load_w(uv_ext, 0, "uv"),
            load_w(av_ext, 0, "av"),
            load_w(bv_ext, 0, "bv"),
        ]
        load_d(d2_ext, d2_sb)
        load_d(d4_ext, d4_sb)
        load_d(d2r_ext, d2r_sb)
        nc.sync.dma_start(rot_sb[:], rot_ext[:, :])

        def fold(t):
            """pass-2 level-2 fold + rotation for c''-block t (needs pass-1
            cbs t, 4+t, 8+t, 12+t done)."""
            p_ = s_pool.tile([P, N], BF16, tag="fp", name="fp")
            q_ = s_pool.tile([P, N], BF16, tag="fq", name="fq")
            nc.gpsimd.tensor_add(p_[:], t_sb[t][:], t_sb[8 + t][:])
            nc.gpsimd.tensor_add(q_[:], t_sb[4 + t][:], t_sb[12 + t][:])
            nc.gpsimd.tensor_add(uu2[t][:], p_[:], q_[:])
            nc.gpsimd.tensor_sub(uv2[t][:], p_[:], q_[:])
            d1 = s_pool.tile([P, N], BF16, tag="fd1", name="fd1")
            d2_ = s_pool.tile([P, N], BF16, tag="fd2", name="fd2")
            nc.vector.tensor_sub(d1[:], t_sb[t][:], t_sb[8 + t][:])
            nc.vector.tensor_sub(d2_[:], t_sb[4 + t][:], t_sb[12 + t][:])
            t1 = s_pool.tile([P, N], BF16, tag="ft1", name="ft1")
            nc.vector.tensor_scalar_mul(t1[:], d2_[:], rot_sb[:, 4 * t + 1 : 4 * t + 2])
            nc.vector.scalar_tensor_tensor(
                av2[t][:], d1[:], rot_sb[:, 4 * t : 4 * t + 1], t1[:],
                AOT.mult, AOT.add,
            )
            t2 = s_pool.tile([P, N], BF16, tag="ft2", name="ft2")
            nc.vector.tensor_scalar_mul(t2[:], d2_[:], rot_sb[:, 4 * t + 3 : 4 * t + 4])
            nc.vector.scalar_tensor_tensor(
                bv2[t][:], d1[:], rot_sb[:, 4 * t + 2 : 4 * t + 3], t2[:],
                AOT.mult, AOT.add,
            )

        # ---- pass 1 ----
        for cb in range(KT):
            ws = w0 if cb == 0 else [
                load_w(uu_ext, cb, "uu"),
                load_w(uv_ext, cb, "uv"),
                load_w(av_ext, cb, "av"),
                load_w(bv_ext, cb, "bv"),
            ]
            dsbs = (d2_sb, d4_sb, d2_sb, d2r_sb)
            pts = []
            for i, (w, dsb) in enumerate(zip(ws, dsbs)):
                pt = ps.tile([P, Qm], F32, tag=f"c{i}", name=f"pt{i}")
                for k in range(4):
                    nc.tensor.matmul(
                        pt[:],
                        lhsT=w[:, k * P : (k + 1) * P],
                        rhs=dsb[k][:],
                        start=(k == 0),
                        stop=(k == 3),
                    )
                pts.append(pt)
            puu, puv, pc, psv = pts
            tcur = t_sb[cb]
            nc.scalar.copy(tcur[:, 0:Qm], puu[:])
            nc.scalar.copy(tcur[:, Qm : 2 * Qm], puv[:])
            nc.scalar.copy(tcur[:, 2 * Qm : 2 * Qm + 1], pc[:, 0:1])
            sv = s_pool.tile([P, Qm], F32, tag="sv", name="sv")
            nc.scalar.copy(sv[:], psv[:])
            nc.vector.tensor_add(
                tcur[:, 2 * Qm + 1 : 3 * Qm], pc[:, 1:Qm], sv[:, 0 : Qm - 1]
            )
            nc.vector.tensor_sub(
                tcur[:, 3 * Qm : 4 * Qm - 1], pc[:, 1:Qm], sv[:, 0 : Qm - 1]
            )
            nc.scalar.mul(tcur[:, 4 * Qm - 1 : 4 * Qm], sv[:, Qm - 1 : Qm], -1.0)
            if cb >= 12:
                fold(cb - 12)

        # ---- pass 2 ----
        off = (0, 2, 1, 3)
        for fb in range(KT):
            srcs = (uu2, uv2, av2, bv2)
            dsbs = (d2_sb, d4_sb, d2_sb, d2r_sb)
            pts = []
            for i, (src, dsb) in enumerate(zip(srcs, dsbs)):
                pt = ps.tile([P, Qm], F32, tag=f"c{i}", name=f"zt{i}")
                for k in range(4):
                    nc.tensor.matmul(
                        pt[:],
                        lhsT=src[k][:, fb * P : (fb + 1) * P],
                        rhs=dsb[k][:],
                        start=(k == 0),
                        stop=(k == 3),
                    )
                pts.append(pt)
            puu, puv, pc, psv = pts
            zt = z_pool.tile([P, N], F32, tag="z", name="zt")
            nc.scalar.copy(zt[:, 0 : N : 4], puu[:])
            nc.scalar.copy(zt[:, 2 : N : 4], puv[:])
            nc.scalar.copy(zt[:, 1:2], pc[:, 0:1])
            sv = s_pool.tile([P, Qm], F32, tag="sv", name="sv")
            nc.scalar.copy(sv[:], psv[:])
            nc.vector.tensor_add(
                zt[:, 5 : N : 4], pc[:, 1:Qm], sv[:, 0 : Qm - 1]
            )
            nc.vector.tensor_sub(
                zt[:, 3 : N - 3 : 4], pc[:, 1:Qm], sv[:, 0 : Qm - 1]
            )
            nc.scalar.mul(zt[:, N - 1 : N], sv[:, Qm - 1 : Qm], -1.0)
            row0 = 4 * P * (fb % 4) + off[fb // 4]
            dma_eng = nc.scalar if fb % 2 == 0 else nc.sync
            dma_eng.dma_start(z_ext[row0 : row0 + 4 * P - 3 : 4, :], zt[:])

    nc.finalize()
    return nc


def _build_split() -> bass.Bass:
    """hi/lo bf16 decomposition: each logical matmul = 3 bf16 matmuls
    (Xh Dh + Xh Dl + Xl Dh), accumulated in the same PSUM chain."""
    nc = bacc.Bacc(None, target_bir_lowering=False)
    xh_ext = nc.declare_dram_parameter("xh", [N, N], BF16, isOutput=False)
    xl_ext = nc.declare_dram_parameter("xl", [N, N], BF16, isOutput=False)
    dh_ext = nc.declare_dram_parameter("dh", [N, N], BF16, isOutput=False)
    dl_ext = nc.declare_dram_parameter("dl", [N, N], BF16, isOutput=False)
    z_ext = nc.declare_dram_parameter("z", [N, N], F32, isOutput=True)

    with ExitStack() as ctx:
        tc = ctx.enter_context(tile.TileContext(nc))
        d_pool = ctx.enter_context(tc.tile_pool(name="d", bufs=1))
        x_pool = ctx.enter_context(tc.tile_pool(name="x", bufs=3))
        w_pool = ctx.enter_context(tc.tile_pool(name="w", bufs=3))
        t_pool = ctx.enter_context(tc.tile_pool(name="t", bufs=KT))
        z_pool = ctx.enter_context(tc.tile_pool(name="z", bufs=3))
        ps1 = ctx.enter_context(tc.tile_pool(name="ps1", bufs=4, space="PSUM"))
        ps2 = ctx.enter_context(tc.tile_pool(name="ps2", bufs=4, space="PSUM"))

        dh_sb = [
            d_pool.tile([P, N], BF16, tag=f"dh{t}", name=f"dh{t}")
            for t in range(KT)
        ]
        dl_sb = [
            d_pool.tile([P, N], BF16, tag=f"dl{t}", name=f"dl{t}")
            for t in range(KT)
        ]
        for fcol in range(NFC):
            for t in range(KT):
                nc.sync.dma_start(
                    dh_sb[t][:, fcol * FC : (fcol + 1) * FC],
                    dh_ext[t * P : (t + 1) * P, fcol * FC : (fcol + 1) * FC],
                )
                nc.sync.dma_start(
                    dl_sb[t][:, fcol * FC : (fcol + 1) * FC],
                    dl_ext[t * P : (t + 1) * P, fcol * FC : (fcol + 1) * FC],
                )

        for fc in range(NFC):
            t_tiles = []
            for cb in range(KT):
                xht = x_pool.tile([P, N], BF16, tag="xh", name="xht")
                xlt = x_pool.tile([P, N], BF16, tag="xl", name="xlt")
                for t_, ext in ((xht, xh_ext), (xlt, xl_ext)):
                    nc.sync.dma_start(
                        t_[:].rearrange("p (t m) -> p t m", t=KT),
                        ext[:, cb * P : (cb + 1) * P].rearrange(
                            "(t p) m -> p t m", p=P
                        ),
                    )
                pt = ps1.tile([P, FC], F32, tag="ps1", name="pt")
                nmm = 3 * KT
                i = 0
                for rt in range(KT):
                    dh = dh_sb[rt][:, fc * FC : (fc + 1) * FC]
                    dl = dl_sb[rt][:, fc * FC : (fc + 1) * FC]
                    xh = xht[:, rt * P : (rt + 1) * P]
                    xl = xlt[:, rt * P : (rt + 1) * P]
                    for l_, r_ in ((xh, dh), (xh, dl), (xl, dh)):
                        nc.tensor.matmul(
                            pt[:], lhsT=l_, rhs=r_,
                            start=(i == 0), stop=(i == nmm - 1),
                        )
                        i += 1
                # split T on device: th = bf16(T), tl = bf16(T - th)
                th = t_pool.tile([P, FC], BF16, tag="th", name="th")
                tl = t_pool.tile([P, FC], BF16, tag="tl", name="tl")
                tmp = w_pool.tile([P, FC], F32, tag="tmp", name="tmp")
                nc.vector.tensor_copy(th[:], pt[:])
                nc.scalar.copy(tmp[:], th[:])
                nc.vector.tensor_sub(tmp[:], pt[:], tmp[:])
                nc.vector.tensor_copy(tl[:], tmp[:])
                t_tiles.append((th, tl))

            for fb in range(FC // P):
                for g in range(NFC):
                    pz = ps2.tile([P, FC], F32, tag="ps2", name="pz")
                    nmm = 3 * KT
                    i = 0
                    for ct in range(KT):
                        th, tl = t_tiles[ct]
                        dh = dh_sb[ct][:, g * FC : (g + 1) * FC]
                        dl = dl_sb[ct][:, g * FC : (g + 1) * FC]
                        thb = th[:, fb * P : (fb + 1) * P]
                        tlb = tl[:, fb * P : (fb + 1) * P]
                        for l_, r_ in ((thb, dh), (thb, dl), (tlb, dh)):
                            nc.tensor.matmul(
                                pz[:], lhsT=l_, rhs=r_,
                                start=(i == 0), stop=(i == nmm - 1),
                            )
                            i += 1
                    zt = z_pool.tile([P, FC], F32, tag="z", name="zt")
                    nc.vector.tensor_copy(zt[:], pz[:])
                    row0 = (fc * (FC // P) + fb) * P
                    nc.sync.dma_start(
                        z_ext[row0 : row0 + P, g * FC : (g + 1) * FC], zt[:]
                    )

    nc.finalize()
    return nc


_PROGRAM_CACHE: dict = {}


_BUILDERS = {
    "f32r": _build_f32r,
    "bfly": _build_bfly,
    "bfly16": _build_bfly16,
    "lvl2": _build_lvl2,
    "split": _build_split,
}


def _mu_perm() -> np.ndarray:
    """perm[128*s + p] = original c index stored at (block s, partition p).
    Blocks 0-3 ascending, 4-7 descending (mirror about 1023), 8-15 mirror of
    0-7 about 2047 -> both pass-2 fold levels are partition-aligned."""
    mu = np.empty((KT, P), np.int64)
    for t in range(4):
        mu[t] = 128 * t + np.arange(P)
        mu[4 + t] = 1023 - 128 * t - np.arange(P)
    for t in range(8):
        mu[8 + t] = 2047 - mu[t]
    return mu.reshape(-1)


def _lvl2_in_maps(x: np.ndarray):
    import ml_dtypes

    Qm = FC
    MU = _mu_perm()
    n5 = np.arange(Qm, dtype=np.float64)
    phi = np.pi * (2 * n5 + 1) / (4 * H)
    cphi, sphi = np.cos(phi), np.sin(phi)
    sgn = (-1.0) ** n5

    def mat2(M):
        n = np.arange(M, dtype=np.float64)[:, None]
        k = np.arange(M, dtype=np.float64)[None, :]
        return np.cos(np.pi * (2 * n + 1) * k / (2 * M))

    def mat4(M):
        n = np.arange(M, dtype=np.float64)[:, None]
        k = np.arange(M, dtype=np.float64)[None, :]
        return np.cos(np.pi * (2 * n + 1) * (2 * k + 1) / (4 * M))

    d2 = mat2(Qm).astype(ml_dtypes.bfloat16)
    d4 = mat4(Qm).astype(ml_dtypes.bfloat16)
    d2r = d2[:, ::-1].copy()
    # pass-2 rotation vectors: c'' = 128*t + p, cols 4t+{cos, sin, -sgn*sin,
    # sgn*cos} with sgn = (-1)^(c'')
    rot = np.empty((P, 16), np.float32)
    for t in range(4):
        c2 = (128 * t + np.arange(P)).astype(np.float64)
        ph = np.pi * (2 * c2 + 1) / (4 * H)
        sg = (-1.0) ** c2
        rot[:, 4 * t + 0] = np.cos(ph)
        rot[:, 4 * t + 1] = np.sin(ph)
        rot[:, 4 * t + 2] = -sg * np.sin(ph)
        rot[:, 4 * t + 3] = sg * np.cos(ph)

    maps = []
    for i in range(B):
        xi = np.asarray(x[i], dtype=np.float32)
        xt, xb = xi[:H], xi[N - 1 : H - 1 : -1]
        u, v = xt + xb, xt - xb
        uu = u[:Qm] + u[H - 1 : Qm - 1 : -1]
        uv = u[:Qm] - u[H - 1 : Qm - 1 : -1]
        vt, vb = v[:Qm], v[H - 1 : Qm - 1 : -1]
        av = vt * cphi[:, None] + vb * sphi[:, None]
        bv = sgn[:, None] * (-vt * sphi[:, None] + vb * cphi[:, None])
        m = {
            "uu": uu[:, MU].astype(ml_dtypes.bfloat16),
            "uv": uv[:, MU].astype(ml_dtypes.bfloat16),
            "av": av[:, MU].astype(ml_dtypes.bfloat16),
            "bv": bv[:, MU].astype(ml_dtypes.bfloat16),
            "d2": d2, "d4": d4, "d2r": d2r, "rot": rot,
        }
        maps.append(m)
    return maps


def _get_program(mode: str) -> bass.Bass:
    if mode not in _PROGRAM_CACHE:
        _PROGRAM_CACHE[mode] = _BUILDERS[mode]()
    return _PROGRAM_CACHE[mode]


def _make_in_maps(x: np.ndarray, mode: str):
    if mode == "lvl2":
        return _lvl2_in_maps(x)
    if mode == "f32r":
        dr = _round_f32r(_dct_matrix_d())
        return [{"x": _round_f32r(x[i]), "d": dr} for i in range(B)]
    if mode == "bfly16":
        import ml_dtypes

        n2 = np.arange(H, dtype=np.float64)[:, None]
        j2 = np.arange(H, dtype=np.float64)[None, :]
        de = np.cos(np.pi * (2 * n2 + 1) * j2 / N).astype(ml_dtypes.bfloat16)
        do = np.cos(np.pi * (2 * n2 + 1) * (2 * j2 + 1) / (2 * N)).astype(
            ml_dtypes.bfloat16
        )
        maps = []
        for i in range(B):
            xi = np.asarray(x[i], dtype=np.float32)
            xr = xi[::-1]
            u = (xi[:H] + xr[:H]).astype(ml_dtypes.bfloat16)
            v = (xi[:H] - xr[:H]).astype(ml_dtypes.bfloat16)
            # Column-reverse blocks 8..15 so pass-1 bottom-half outputs come
            # out partition-mirrored (see _build_bfly16 pass-2 fold).
            for w in (u, v):
                blk = w[:, H:].reshape(H, KT2, P)
                w[:, H:] = blk[:, :, ::-1].reshape(H, H)
            maps.append({"u": u, "v": v, "de": de, "do": do})
        return maps
    if mode == "bfly":
        n2 = np.arange(H, dtype=np.float64)[:, None]
        j2 = np.arange(H, dtype=np.float64)[None, :]
        de = _round_f32r(np.cos(np.pi * (2 * n2 + 1) * j2 / N).astype(np.float32))
        do = _round_f32r(
            np.cos(np.pi * (2 * n2 + 1) * (2 * j2 + 1) / (2 * N)).astype(
                np.float32
            )
        )
        maps = []
        for i in range(B):
            xi = np.asarray(x[i], dtype=np.float32)
            xr = xi[::-1]
            u = _round_f32r(xi[:H] + xr[:H])
            v = _round_f32r(xi[:H] - xr[:H])
            # Column-reverse blocks 8..15: pass-1 output partitions for those
            # blocks then come out mirror-ordered, which makes the pass-2
            # bottom-half fold a plain ascending read (see _build_bfly).
            for w in (u, v):
                blk = w[:, H:].reshape(H, KT2, P)
                w[:, H:] = blk[:, :, ::-1].reshape(H, H)
            maps.append({"u": u, "v": v, "de": de, "do": do})
        return maps
    dh, dl = _split_bf16(_dct_matrix_d())
    maps = []
    for i in range(B):
        xh, xl = _split_bf16(np.ascontiguousarray(x[i], dtype=np.float32))
        maps.append({"xh": xh, "xl": xl, "dh": dh, "dl": dl})
    return maps


def kernel(x: np.ndarray) -> np.ndarray:
    x = np.asarray(x)
    assert x.shape == (B, N, N), x.shape
    nc = _get_program(MODE)
    in_maps = _make_in_maps(x, MODE)
    res = run_bass_kernel_spmd(nc, in_maps, list(range(B)))
    out = np.stack([res.results[i]["z"] for i in range(B)], axis=0)
    return out.astype(np.float32, copy=False)



# revision 19
# speedup vs baseline: 1.6281x; 1.3225x over previous
"""Trainium2 Bass kernel: batched 2D DCT-II (unnormalized), x: (8, 2048, 2048) f32.

Math: per image X, the unnormalized 2D DCT-II is Z = C @ X @ C^T with
C[k,n] = cos(pi*(2n+1)*k/(2N)).  Let D = C^T.  Using the PE's
out = lhsT.T @ rhs semantics with the *data* as the stationary operand and D as
the moving operand, the two passes chain with no transposes:
    pass 1:  T = X^T @ D      (lhsT = X tiles,  rhs = D)   -> T[c, f]
    pass 2:  Z = T^T @ D      (lhsT = T tiles,  rhs = D)   -> Z = D^T X D = C X C^T

Sharding: batch dim 8 -> one image per NeuronCore (data parallel, no comms).

Dtype modes:
  "f32r"  - single fp32r matmul per term (full PE rate; TF32-like 11-bit
            mantissa operand rounding; ~2e-4 relative-to-absmax error).
  "split" - hi/lo bf16 decomposition, 3 matmuls per term (~5e-6 error, 3x cost).
"""

import numpy as np
from contextlib import ExitStack

import concourse.bass as bass
import concourse.bacc as bacc
import concourse.tile as tile
from concourse import mybir
from concourse.bass_utils import run_bass_kernel_spmd

F32 = mybir.dt.float32
F32R = mybir.dt.float32r
BF16 = mybir.dt.bfloat16

import os

MODE = os.environ.get("DCT_MODE", "lvl2")  # "lvl2", "bfly16", "bfly", "f32r", "split"

B = 8          # batch == n_cores
N = 2048       # image is N x N
P = 128        # partitions
KT = N // P    # 16 k-tiles along any contraction
FC = 512       # chunk width (pass-1 f-chunk, pass-2 g-chunk, PSUM bank)
NFC = N // FC  # 4 chunks
H = N // 2     # butterfly half size
KT2 = H // P   # 8 k-tiles at half contraction


def _round_f32r(a: np.ndarray) -> np.ndarray:
    """fp32r = round-to-nearest, 11 explicit mantissa bits (drop low 12)."""
    b = np.ascontiguousarray(a, dtype=np.float32).view(np.uint32)
    r = ((b + np.uint32(0x800)) & np.uint32(0xFFFFF000)).view(np.float32)
    return r


def _split_bf16(a: np.ndarray):
    import ml_dtypes

    hi = a.astype(ml_dtypes.bfloat16)
    lo = (a - hi.astype(np.float32)).astype(ml_dtypes.bfloat16)
    return hi, lo


def _dct_matrix_d() -> np.ndarray:
    # D[n, k] = cos(pi * (2n+1) * k / (2N)), exact in float64
    n = np.arange(N, dtype=np.float64)[:, None]
    k = np.arange(N, dtype=np.float64)[None, :]
    d = np.cos(np.pi * (2.0 * n + 1.0) * k / (2.0 * N))
    return d.astype(np.float32)


def _build_f32r() -> bass.Bass:
    """fp32r two-pass DCT with the intermediate T round-tripped via DRAM.

    Pass 1 streams X once (one column-block per chain, all 4 f-chunks while
    the block is resident).  T chunks are written back to a DRAM scratch and
    re-streamed as pass-2 stationary tiles.  D stays resident in SBUF.
    """
    nc = bacc.Bacc(None, target_bir_lowering=False)
    x_ext = nc.declare_dram_parameter("x", [N, N], F32R, isOutput=False)
    d_ext = nc.declare_dram_parameter("d", [N, N], F32R, isOutput=False)
    z_ext = nc.declare_dram_parameter("z", [N, N], F32, isOutput=True)

    with ExitStack() as ctx:
        tc = ctx.enter_context(tile.TileContext(nc))
        d_pool = ctx.enter_context(tc.tile_pool(name="d", bufs=1))
        x_pool = ctx.enter_context(tc.tile_pool(name="x", bufs=3))
        t_pool = ctx.enter_context(tc.tile_pool(name="t", bufs=6))
        z_pool = ctx.enter_context(tc.tile_pool(name="z", bufs=3))
        dram = ctx.enter_context(tc.tile_pool(name="dram", bufs=1, space="DRAM"))
        ps1 = ctx.enter_context(tc.tile_pool(name="ps1", bufs=4, space="PSUM"))
        ps2 = ctx.enter_context(tc.tile_pool(name="ps2", bufs=4, space="PSUM"))

        t_dram = dram.tile([N, N], F32R, name="t_dram")

        # First column-block of X loads before D so pass 1 starts early.
        d_sb = [
            d_pool.tile([P, N], F32R, tag=f"d{t}", name=f"d{t}") for t in range(KT)
        ]

        def load_x(cb):
            xt = x_pool.tile([P, N], F32R, tag="x", name="xt")
            nc.sync.dma_start(
                xt[:].rearrange("p (t m) -> p t m", t=KT),
                x_ext[:, cb * P : (cb + 1) * P].rearrange("(t p) m -> p t m", p=P),
            )
            return xt

        x0 = load_x(0)
        # D f-chunk 0 for all 16 row-tiles (pass-1 chain 0 needs only these)
        for fcol in range(NFC):
            for t in range(KT):
                nc.sync.dma_start(
                    d_sb[t][:, fcol * FC : (fcol + 1) * FC],
                    d_ext[t * P : (t + 1) * P, fcol * FC : (fcol + 1) * FC],
                )
            if fcol == 0:
                # remaining D chunks stream behind pass-1 compute
                pass

        # ---- pass 1: per column-block cb, all f-chunks: T[cb,:] = (X^T D)[cb,:]
        for cb in range(KT):
            xt = x0 if cb == 0 else load_x(cb)
            for fc in range(NFC):
                pt = ps1.tile([P, FC], F32, tag="ps1", name="pt")
                for rt in range(KT):
                    nc.tensor.matmul(
                        pt[:],
                        lhsT=xt[:, rt * P : (rt + 1) * P],
                        rhs=d_sb[rt][:, fc * FC : (fc + 1) * FC],
                        start=(rt == 0),
                        stop=(rt == KT - 1),
                    )
                tt = t_pool.tile([P, FC], F32R, tag="t", name="tt")
                nc.vector.tensor_copy(tt[:], pt[:])
                nc.scalar.dma_start(
                    t_dram[cb * P : (cb + 1) * P, fc * FC : (fc + 1) * FC], tt[:]
                )

        # ---- pass 2: per f-block fb: Z[fb,:] = (T^T D)[fb,:]
        for fb in range(KT):
            tf = x_pool.tile([P, N], F32R, tag="x", name="tf")
            nc.sync.dma_start(
                tf[:].rearrange("p (t m) -> p t m", t=KT),
                t_dram[:, fb * P : (fb + 1) * P].rearrange("(t p) m -> p t m", p=P),
            )
            for g in range(NFC):
                pz = ps2.tile([P, FC], F32, tag="ps2", name="pz")
                for ct in range(KT):
                    nc.tensor.matmul(
                        pz[:],
                        lhsT=tf[:, ct * P : (ct + 1) * P],
                        rhs=d_sb[ct][:, g * FC : (g + 1) * FC],
                        start=(ct == 0),
                        stop=(ct == KT - 1),
                    )
                zt = z_pool.tile([P, FC], F32, tag="z", name="zt")
                nc.vector.tensor_copy(zt[:], pz[:])
                nc.scalar.dma_start(
                    z_ext[fb * P : (fb + 1) * P, g * FC : (g + 1) * FC], zt[:]
                )

    nc.finalize()
    return nc


def _build_bfly() -> bass.Bass:
    """Radix-2 even/odd DCT factorization in fp32r: each 1D DCT-II of size N
    becomes two size-N/2 cosine transforms of the folded sequences
    u = x_top + reverse(x_bot), v = x_top - reverse(x_bot):
        y[2j]   = sum_{n<H} u[n] De[n, j],   De[n,j] = cos(pi (2n+1) j / N)
        y[2j+1] = sum_{n<H} v[n] Do[n, j],   Do[n,j] = cos(pi (2n+1)(2j+1) / 2N)
    Halves the matmul work per pass.  Pass-1 folding is done on the host
    (u/v uploaded); pass-2 folding of the intermediate T is done by DVE with a
    reversed-row DMA load.  Outputs are de-interleaved on chip (strided DVE
    writes) + stride-2-row DMA stores, so all DRAM traffic stays contiguous
    per partition.
    """
    nc = bacc.Bacc(None, target_bir_lowering=False)
    u_ext = nc.declare_dram_parameter("u", [H, N], F32R, isOutput=False)
    v_ext = nc.declare_dram_parameter("v", [H, N], F32R, isOutput=False)
    de_ext = nc.declare_dram_parameter("de", [H, H], F32R, isOutput=False)
    do_ext = nc.declare_dram_parameter("do", [H, H], F32R, isOutput=False)
    z_ext = nc.declare_dram_parameter("z", [N, N], F32, isOutput=True)

    with ExitStack() as ctx:
        tc = ctx.enter_context(tile.TileContext(nc))
        d_pool = ctx.enter_context(tc.tile_pool(name="d", bufs=1))
        x_pool = ctx.enter_context(tc.tile_pool(name="x", bufs=4))
        t_pool = ctx.enter_context(tc.tile_pool(name="t", bufs=4))
        b_pool = ctx.enter_context(tc.tile_pool(name="b", bufs=4))
        z_pool = ctx.enter_context(tc.tile_pool(name="z", bufs=3))
        dram = ctx.enter_context(tc.tile_pool(name="dram", bufs=1, space="DRAM"))
        # PSUM: pass-1 accumulators 2x1 bank; pass-2 output chains share one
        # 6-slot pool (6 banks) so slow de-interleave copies never stall the
        # next chain. 8 banks total (no reversal matmuls anymore).
        ps1 = ctx.enter_context(tc.tile_pool(name="ps1", bufs=2, space="PSUM"))
        ps2 = ctx.enter_context(tc.tile_pool(name="ps2", bufs=6, space="PSUM"))

        # T in blocked layout: cols [0,H) = even outputs, [H,2H) = odd
        t_dram = dram.tile([N, N], F32R, name="t_dram")

        de_sb = [
            d_pool.tile([P, H], F32R, tag=f"de{t}", name=f"de{t}")
            for t in range(KT2)
        ]
        do_sb = [
            d_pool.tile([P, H], F32R, tag=f"do{t}", name=f"do{t}")
            for t in range(KT2)
        ]

        def load_block(ext, cb, tag):
            w = x_pool.tile([P, H], F32R, tag=tag, name="w_" + tag)
            nc.sync.dma_start(
                w[:].rearrange("p (t m) -> p t m", t=KT2),
                ext[:, cb * P : (cb + 1) * P].rearrange("(t p) m -> p t m", p=P),
            )
            return w

        # loads in exact first-consumption order: u0, de jc0, de jc1, v0,
        # do jc0, do jc1; the pass-2 reversal matrix r last.
        u0 = load_block(u_ext, 0, "u")
        for jc in range(2):
            for t in range(KT2):
                nc.sync.dma_start(
                    de_sb[t][:, jc * FC : (jc + 1) * FC],
                    de_ext[t * P : (t + 1) * P, jc * FC : (jc + 1) * FC],
                )
        v0 = load_block(v_ext, 0, "v")
        for jc in range(2):
            for t in range(KT2):
                nc.sync.dma_start(
                    do_sb[t][:, jc * FC : (jc + 1) * FC],
                    do_ext[t * P : (t + 1) * P, jc * FC : (jc + 1) * FC],
                )

        # ---- pass 1: T_blk[cb, :] ----
        for cb in range(KT):
            ut = u0 if cb == 0 else load_block(u_ext, cb, "u")
            vt = v0 if cb == 0 else load_block(v_ext, cb, "v")
            for half, (wt, dsb) in enumerate(((ut, de_sb), (vt, do_sb))):
                for jc in range(2):
                    pt = ps1.tile([P, FC], F32, tag="acc", name="pt")
                    for rt in range(KT2):
                        nc.tensor.matmul(
                            pt[:],
                            lhsT=wt[:, rt * P : (rt + 1) * P],
                            rhs=dsb[rt][:, jc * FC : (jc + 1) * FC],
                            start=(rt == 0),
                            stop=(rt == KT2 - 1),
                        )
                    tt = t_pool.tile([P, FC], F32R, tag="t", name="tt")
                    nc.vector.tensor_copy(tt[:], pt[:])
                    col0 = half * H + jc * FC
                    # Bottom-half blocks (cb>=8) arrive partition-reversed
                    # (host reversed their lhsT columns) and are stored
                    # mirror-ordered: row 1024+k holds T[2047-k].  Then the
                    # pass-2 fold reads both halves with plain ascending loads.
                    row0 = cb * P if cb < KT2 else (23 * P - cb * P)
                    nc.scalar.dma_start(
                        t_dram[row0 : row0 + P, col0 : col0 + FC], tt[:]
                    )

        # ---- pass 2: fold T over rows, transform, de-interleave out ----
        # bot_rev[c', f] = T[2047-c', f]: partition reversal via one PE matmul
        # with the reversal permutation R (out[m,n] = bot[127-m, n]); the
        # tile-order flip (ct -> 7-ct) via a reversed free-dim view in the add.
        # Software-pipelined: loads run 3 blocks ahead, reversal matmul + DVE
        # fold 2 ahead, so block fb's chains never wait on its fold.
        folded: dict = {}

        def p2_load(fb):
            top = b_pool.tile([P, H], F32R, tag="top", name="top")
            nc.sync.dma_start(
                top[:].rearrange("p (t m) -> p t m", t=KT2),
                t_dram[0:H, fb * P : (fb + 1) * P].rearrange(
                    "(t p) m -> p t m", p=P
                ),
            )
            bot = b_pool.tile([P, H], F32R, tag="bot", name="bot")
            nc.sync.dma_start(
                bot[:].rearrange("p (t m) -> p t m", t=KT2),
                t_dram[H:N, fb * P : (fb + 1) * P].rearrange(
                    "(t p) m -> p t m", p=P
                ),
            )
            folded[fb] = (top, bot)

        def p2_fold(fb):
            # mirror-ordered bottom storage makes the fold a plain 2D add/sub
            top, bot = folded[fb]
            u2 = b_pool.tile([P, H], F32R, tag="u2", name="u2")
            nc.vector.tensor_add(u2[:], top[:], bot[:])
            v2 = b_pool.tile([P, H], F32R, tag="v2", name="v2")
            nc.vector.tensor_sub(v2[:], top[:], bot[:])
            folded[fb] = (u2, v2)

        p2_load(0)
        p2_load(1)
        p2_fold(0)
        p2_load(2)
        p2_fold(1)
        for fb in range(KT):
            u2, v2 = folded.pop(fb)
            # f_blk block fb -> actual Z rows (de-interleave rows via stride 2)
            if fb < KT2:
                row0 = 2 * fb * P
                row_stop = row0 + 2 * P
            else:
                row0 = 2 * (fb - KT2) * P + 1
                row_stop = row0 + 2 * P - 1
            for jc in range(2):
                pe_ = ps2.tile([P, FC], F32, tag="zacc", name="pe_")
                for ct in range(KT2):
                    nc.tensor.matmul(
                        pe_[:],
                        lhsT=u2[:, ct * P : (ct + 1) * P],
                        rhs=de_sb[ct][:, jc * FC : (jc + 1) * FC],
                        start=(ct == 0),
                        stop=(ct == KT2 - 1),
                    )
                po_ = ps2.tile([P, FC], F32, tag="zacc", name="po_")
                for ct in range(KT2):
                    nc.tensor.matmul(
                        po_[:],
                        lhsT=v2[:, ct * P : (ct + 1) * P],
                        rhs=do_sb[ct][:, jc * FC : (jc + 1) * FC],
                        start=(ct == 0),
                        stop=(ct == KT2 - 1),
                    )
                zt = z_pool.tile([P, 2 * FC], F32, tag="z", name="zt")
                nc.scalar.copy(zt[:, 0 : 2 * FC : 2], pe_[:])
                nc.vector.tensor_copy(zt[:, 1 : 2 * FC : 2], po_[:])
                nc.scalar.dma_start(
                    z_ext[row0:row_stop:2, jc * 2 * FC : (jc + 1) * 2 * FC],
                    zt[:],
                )
            if fb + 3 < KT:
                p2_load(fb + 3)
            if fb + 2 < KT:
                p2_fold(fb + 2)

    nc.finalize()
    return nc


def _build_bfly16() -> bass.Bass:
    """bf16 radix-2 butterfly DCT with the intermediate T kept entirely in
    SBUF (no DRAM roundtrip).  Same math/layout as _build_bfly: pass-1 folding
    host-side (u/v uploaded, bottom column-blocks mirrored), blocked T
    [even|odd], pass-2 fold via partition-aligned mirror blocks, outputs
    de-interleaved on chip + stride-2 row DMA stores.  bf16 operands double
    the PE rate vs fp32r and halve SBUF/DMA footprint; PSUM accumulates f32.
    """
    nc = bacc.Bacc(None, target_bir_lowering=False)
    u_ext = nc.declare_dram_parameter("u", [H, N], BF16, isOutput=False)
    v_ext = nc.declare_dram_parameter("v", [H, N], BF16, isOutput=False)
    de_ext = nc.declare_dram_parameter("de", [H, H], BF16, isOutput=False)
    do_ext = nc.declare_dram_parameter("do", [H, H], BF16, isOutput=False)
    z_ext = nc.declare_dram_parameter("z", [N, N], F32, isOutput=True)

    with ExitStack() as ctx:
        tc = ctx.enter_context(tile.TileContext(nc))
        d_pool = ctx.enter_context(tc.tile_pool(name="d", bufs=1))
        x_pool = ctx.enter_context(tc.tile_pool(name="x", bufs=4))
        t_pool = ctx.enter_context(tc.tile_pool(name="t", bufs=1))
        f_pool = ctx.enter_context(tc.tile_pool(name="f", bufs=1))
        z_pool = ctx.enter_context(tc.tile_pool(name="z", bufs=3))
        ps1 = ctx.enter_context(tc.tile_pool(name="ps1", bufs=2, space="PSUM"))
        ps2 = ctx.enter_context(tc.tile_pool(name="ps2", bufs=6, space="PSUM"))

        de_sb = [
            d_pool.tile([P, H], BF16, tag=f"de{t}", name=f"de{t}")
            for t in range(KT2)
        ]
        do_sb = [
            d_pool.tile([P, H], BF16, tag=f"do{t}", name=f"do{t}")
            for t in range(KT2)
        ]
        # T resident in SBUF: storage block s holds pass-1 output of
        # cb = s (s<8) or cb = 23-s (s>=8, partition-mirrored rows).
        t_sb = [
            t_pool.tile([P, N], BF16, tag=f"t{s}", name=f"t{s}")
            for s in range(KT)
        ]

        def load_block(ext, cb, tag):
            w = x_pool.tile([P, H], BF16, tag=tag, name="w_" + tag)
            nc.sync.dma_start(
                w[:].rearrange("p (t m) -> p t m", t=KT2),
                ext[:, cb * P : (cb + 1) * P].rearrange("(t p) m -> p t m", p=P),
            )
            return w

        u0 = load_block(u_ext, 0, "u")
        for jc in range(2):
            for t in range(KT2):
                nc.sync.dma_start(
                    de_sb[t][:, jc * FC : (jc + 1) * FC],
                    de_ext[t * P : (t + 1) * P, jc * FC : (jc + 1) * FC],
                )
        v0 = load_block(v_ext, 0, "v")
        for jc in range(2):
            for t in range(KT2):
                nc.sync.dma_start(
                    do_sb[t][:, jc * FC : (jc + 1) * FC],
                    do_ext[t * P : (t + 1) * P, jc * FC : (jc + 1) * FC],
                )

        # ---- pass 1: T_blk[cb, :] straight into SBUF ----
        for cb in range(KT):
            ut = u0 if cb == 0 else load_block(u_ext, cb, "u")
            vt = v0 if cb == 0 else load_block(v_ext, cb, "v")
            s = cb if cb < KT2 else 23 - cb
            for half, (wt, dsb) in enumerate(((ut, de_sb), (vt, do_sb))):
                for jc in range(2):
                    pt = ps1.tile([P, FC], F32, tag="acc", name="pt")
                    for rt in range(KT2):
                        nc.tensor.matmul(
                            pt[:],
                            lhsT=wt[:, rt * P : (rt + 1) * P],
                            rhs=dsb[rt][:, jc * FC : (jc + 1) * FC],
                            start=(rt == 0),
                            stop=(rt == KT2 - 1),
                        )
                    col0 = half * H + jc * FC
                    nc.vector.tensor_copy(
                        t_sb[s][:, col0 : col0 + FC], pt[:]
                    )

        # ---- pass 2: fold T in SBUF, transform, de-interleave out ----
        u2 = [
            f_pool.tile([P, N], BF16, tag=f"u2_{t}", name=f"u2_{t}")
            for t in range(KT2)
        ]
        v2 = [
            f_pool.tile([P, N], BF16, tag=f"v2_{t}", name=f"v2_{t}")
            for t in range(KT2)
        ]
        # fold order t=7..0: pair (t, 8+t) is ready once cb=15-t finished,
        # so later folds wait on earlier pass-1 chains.
        for t in range(KT2 - 1, -1, -1):
            nc.vector.tensor_add(u2[t][:], t_sb[t][:], t_sb[KT2 + t][:])
            nc.vector.tensor_sub(v2[t][:], t_sb[t][:], t_sb[KT2 + t][:])

        for fb in range(KT):
            if fb < KT2:
                row0 = 2 * fb * P
                row_stop = row0 + 2 * P
            else:
                row0 = 2 * (fb - KT2) * P + 1
                row_stop = row0 + 2 * P - 1
            for jc in range(2):
                pe_ = ps2.tile([P, FC], F32, tag="zacc", name="pe_")
                for ct in range(KT2 - 1, -1, -1):
                    nc.tensor.matmul(
                        pe_[:],
                        lhsT=u2[ct][:, fb * P : (fb + 1) * P],
                        rhs=de_sb[ct][:, jc * FC : (jc + 1) * FC],
                        start=(ct == KT2 - 1),
                        stop=(ct == 0),
                    )
                po_ = ps2.tile([P, FC], F32, tag="zacc", name="po_")
                for ct in range(KT2 - 1, -1, -1):
                    nc.tensor.matmul(
                        po_[:],
                        lhsT=v2[ct][:, fb * P : (fb + 1) * P],
                        rhs=do_sb[ct][:, jc * FC : (jc + 1) * FC],
                        start=(ct == KT2 - 1),
                        stop=(ct == 0),
                    )
                zt = z_pool.tile([P, 2 * FC], F32, tag="z", name="zt")
                nc.scalar.copy(zt[:, 0 : 2 * FC : 2], pe_[:])
                nc.vector.tensor_copy(zt[:, 1 : 2 * FC : 2], po_[:])
                nc.scalar.dma_start(
                    z_ext[row0:row_stop:2, jc * 2 * FC : (jc + 1) * 2 * FC],
                    zt[:],
                )

    nc.finalize()
    return nc


def _build_lvl2() -> bass.Bass:
    """Level-2 pole-free butterfly DCT (bf16, T in SBUF).

    1D DCT-II_2048 factored twice:
      fold1: u = xt + xb_rev (DCT-II_1024), v = xt - xb_rev (DCT-IV_1024)
      u: fold2 -> uu (DCT-II_512), uv (DCT-IV_512)
      v (DCT-IV_1024, stable rotation form): av/bv Givens-rotated pairs,
         both through DCT-II_512 (bv with column-reversed matrix), then a
         post-butterfly of adjacent outputs: yodd[2j] = C[j] + S[j],
         yodd[2j+1] = C[j+1] - S[j+1], S[j] = SBV[Q-j].
    Pass-1 folds/rotations on host; pass-2 folds/rotations on device from
    SBUF-resident blocked T.  mu column permutation makes both pass-2 fold
    levels partition-aligned.  PE work is N^3/4 per pass (half of level-1).
    """
    nc = bacc.Bacc(None, target_bir_lowering=False)
    Qm = FC  # 512
    uu_ext = nc.declare_dram_parameter("uu", [Qm, N], BF16, isOutput=False)
    uv_ext = nc.declare_dram_parameter("uv", [Qm, N], BF16, isOutput=False)
    av_ext = nc.declare_dram_parameter("av", [Qm, N], BF16, isOutput=False)
    bv_ext = nc.declare_dram_parameter("bv", [Qm, N], BF16, isOutput=False)
    d2_ext = nc.declare_dram_parameter("d2", [Qm, Qm], BF16, isOutput=False)
    d4_ext = nc.declare_dram_parameter("d4", [Qm, Qm], BF16, isOutput=False)
    d2r_ext = nc.declare_dram_parameter("d2r", [Qm, Qm], BF16, isOutput=False)
    cd2_ext = nc.declare_dram_parameter("cd2", [Qm, Qm], BF16, isOutput=False)
    scd2r_ext = nc.declare_dram_parameter("scd2r", [Qm, Qm], BF16, isOutput=False)
    rot_ext = nc.declare_dram_parameter("rot", [P, 4], F32, isOutput=False)
    z_ext = nc.declare_dram_parameter("z", [N, N], F32, isOutput=True)

    AOT = __import__("concourse.alu_op_type", fromlist=["AluOpType"]).AluOpType

    with ExitStack() as ctx:
        tc = ctx.enter_context(tile.TileContext(nc))
        d_pool = ctx.enter_context(tc.tile_pool(name="d", bufs=1))
        x_pool = ctx.enter_context(tc.tile_pool(name="x", bufs=3))
        t_pool = ctx.enter_context(tc.tile_pool(name="t", bufs=1))
        f_pool = ctx.enter_context(tc.tile_pool(name="f", bufs=1))
        s_pool = ctx.enter_context(tc.tile_pool(name="s", bufs=1))
        z_pool = ctx.enter_context(tc.tile_pool(name="z", bufs=2))
        ps = ctx.enter_context(tc.tile_pool(name="ps", bufs=2, space="PSUM"))

        d2_sb = [d_pool.tile([P, Qm], BF16, tag=f"d2_{k}", name=f"d2_{k}") for k in range(4)]
        d4_sb = [d_pool.tile([P, Qm], BF16, tag=f"d4_{k}", name=f"d4_{k}") for k in range(4)]
        d2r_sb = [d_pool.tile([P, Qm], BF16, tag=f"d2r_{k}", name=f"d2r_{k}") for k in range(4)]
        cd2_sb = [d_pool.tile([P, Qm], BF16, tag=f"cd2_{k}", name=f"cd2_{k}") for k in range(4)]
        scd2r_sb = [d_pool.tile([P, Qm], BF16, tag=f"scd2r_{k}", name=f"scd2r_{k}") for k in range(4)]
        rot_sb = d_pool.tile([P, 4], F32, tag="rot", name="rot")
        t_sb = [t_pool.tile([P, N], BF16, tag=f"t{s}", name=f"t{s}") for s in range(KT)]
        uu2 = [f_pool.tile([P, N], BF16, tag=f"uu2_{t}", name=f"uu2_{t}") for t in range(4)]
        uv2 = [f_pool.tile([P, N], BF16, tag=f"uv2_{t}", name=f"uv2_{t}") for t in range(4)]
        av2 = [f_pool.tile([P, N], BF16, tag=f"av2_{t}", name=f"av2_{t}") for t in range(4)]
        bv2 = [f_pool.tile([P, N], BF16, tag=f"bv2_{t}", name=f"bv2_{t}") for t in range(4)]

        def load_w(ext, cb, tag):
            w = x_pool.tile([P, 4 * P], BF16, tag=tag, name="w_" + tag)
            nc.sync.dma_start(
                w[:].rearrange("p (t m) -> p t m", t=4),
                ext[:, cb * P : (cb + 1) * P].rearrange("(t p) m -> p t m", p=P),
            )
            return w

        def load_d(ext, sb):
            for k in range(4):
                nc.sync.dma_start(sb[k][:], ext[k * P : (k + 1) * P, :])

        w0 = [load_w(uu_ext, 0, "uu")]
        load_d(d2_ext, d2_sb)
        w0.append(load_w(uv_ext, 0, "uv"))
        load_d(d4_ext, d4_sb)
        w0.append(load_w(av_ext, 0, "av"))
        w0.append(load_w(bv_ext, 0, "bv"))
        load_d(d2r_ext, d2r_sb)
        nc.sync.dma_start(rot_sb[:], rot_ext[:, :])
        load_d(cd2_ext, cd2_sb)
        load_d(scd2r_ext, scd2r_sb)

        def fold(t):
            """pass-2 level-2 fold + tan-rotation for c''-block t (needs
            pass-1 cbs t, 4+t, 8+t, 12+t done).  The cos row-scales of the
            rotation live in the cd2/scd2r rhs matrices."""
            p_ = s_pool.tile([P, N], BF16, tag="fp", name="fp")
            q_ = s_pool.tile([P, N], BF16, tag="fq", name="fq")
            nc.vector.tensor_add(p_[:], t_sb[t][:], t_sb[8 + t][:])
            nc.vector.tensor_add(q_[:], t_sb[4 + t][:], t_sb[12 + t][:])
            nc.vector.tensor_add(uu2[t][:], p_[:], q_[:])
            nc.vector.tensor_sub(uv2[t][:], p_[:], q_[:])
            d1 = s_pool.tile([P, N], BF16, tag="fd1", name="fd1")
            d2_ = s_pool.tile([P, N], BF16, tag="fd2", name="fd2")
            nc.vector.tensor_sub(d1[:], t_sb[t][:], t_sb[8 + t][:])
            nc.vector.tensor_sub(d2_[:], t_sb[4 + t][:], t_sb[12 + t][:])
            t1 = s_pool.tile([P, N], BF16, tag="ft1", name="ft1")
            nc.scalar.mul(t1[:], d2_[:], rot_sb[:, t : t + 1])
            nc.vector.tensor_add(av2[t][:], d1[:], t1[:])
            t2 = s_pool.tile([P, N], BF16, tag="ft2", name="ft2")
            nc.scalar.mul(t2[:], d1[:], rot_sb[:, t : t + 1])
            nc.vector.tensor_sub(bv2[t][:], d2_[:], t2[:])

        # ---- pass 1 ----
        cb_order = [t + 4 * i for t in range(4) for i in range(4)]
        for idx, cb in enumerate(cb_order):
            ws = w0 if cb == 0 else [
                load_w(uu_ext, cb, "uu"),
                load_w(uv_ext, cb, "uv"),
                load_w(av_ext, cb, "av"),
                load_w(bv_ext, cb, "bv"),
            ]
            dsbs = (d2_sb, d4_sb, d2_sb, d2r_sb)
            pts = []
            for i, (w, dsb) in enumerate(zip(ws, dsbs)):
                pt = ps.tile([P, Qm], F32, tag=f"c{i}", name=f"pt{i}")
                for k in range(4):
                    nc.tensor.matmul(
                        pt[:],
                        lhsT=w[:, k * P : (k + 1) * P],
                        rhs=dsb[k][:],
                        start=(k == 0),
                        stop=(k == 3),
                    )
                pts.append(pt)
            puu, puv, pc, psv = pts
            tcur = t_sb[cb]
            nc.scalar.copy(tcur[:, 0:Qm], puu[:])
            nc.scalar.copy(tcur[:, Qm : 2 * Qm], puv[:])
            nc.scalar.copy(tcur[:, 2 * Qm : 2 * Qm + 1], pc[:, 0:1])
            sv = s_pool.tile([P, Qm], F32, tag="sv", name="sv")
            nc.scalar.copy(sv[:], psv[:])
            nc.vector.tensor_add(
                tcur[:, 2 * Qm + 1 : 3 * Qm], pc[:, 1:Qm], sv[:, 0 : Qm - 1]
            )
            nc.vector.tensor_sub(
                tcur[:, 3 * Qm : 4 * Qm - 1], pc[:, 1:Qm], sv[:, 0 : Qm - 1]
            )
            nc.scalar.mul(tcur[:, 4 * Qm - 1 : 4 * Qm], sv[:, Qm - 1 : Qm], -1.0)
            if idx % 4 == 3:
                fold(idx // 4)

        # ---- pass 2 ----
        off = (0, 2, 1, 3)
        for fb in range(KT):
            srcs = (uu2, uv2, av2, bv2)
            dsbs = (d2_sb, d4_sb, cd2_sb, scd2r_sb)
            pts = []
            for i, (src, dsb) in enumerate(zip(srcs, dsbs)):
                pt = ps.tile([P, Qm], F32, tag=f"c{i}", name=f"zt{i}")
                for k in range(4):
                    nc.tensor.matmul(
                        pt[:],
                        lhsT=src[k][:, fb * P : (fb + 1) * P],
                        rhs=dsb[k][:],
                        start=(k == 0),
                        stop=(k == 3),
                    )
                pts.append(pt)
            puu, puv, pc, psv = pts
            zt = z_pool.tile([P, N], F32, tag="z", name="zt")
            nc.scalar.copy(zt[:, 0 : N : 4], puu[:])
            nc.scalar.copy(zt[:, 2 : N : 4], puv[:])
            nc.scalar.copy(zt[:, 1:2], pc[:, 0:1])
            sv = s_pool.tile([P, Qm], F32, tag="sv", name="sv")
            nc.scalar.copy(sv[:], psv[:])
            nc.vector.tensor_add(
                zt[:, 5 : N : 4], pc[:, 1:Qm], sv[:, 0 : Qm - 1]
            )
            nc.vector.tensor_sub(
                zt[:, 3 : N - 3 : 4], pc[:, 1:Qm], sv[:, 0 : Qm - 1]
            )
            nc.scalar.mul(zt[:, N - 1 : N], sv[:, Qm - 1 : Qm], -1.0)
            row0 = 4 * P * (fb % 4) + off[fb // 4]
            dma_eng = nc.scalar if fb % 2 == 0 else nc.sync
            dma_eng.dma_start(z_ext[row0 : row0 + 4 * P - 3 : 4, :], zt[:])

    nc.finalize()
    return nc


def _build_split() -> bass.Bass:
    """hi/lo bf16 decomposition: each logical matmul = 3 bf16 matmuls
    (Xh Dh + Xh Dl + Xl Dh), accumulated in the same PSUM chain."""
    nc = bacc.Bacc(None, target_bir_lowering=False)
    xh_ext = nc.declare_dram_parameter("xh", [N, N], BF16, isOutput=False)
    xl_ext = nc.declare_dram_parameter("xl", [N, N], BF16, isOutput=False)
    dh_ext = nc.declare_dram_parameter("dh", [N, N], BF16, isOutput=False)
    dl_ext = nc.declare_dram_parameter("dl", [N, N], BF16, isOutput=False)
    z_ext = nc.declare_dram_parameter("z", [N, N], F32, isOutput=True)

    with ExitStack() as ctx:
        tc = ctx.enter_context(tile.TileContext(nc))
        d_pool = ctx.enter_context(tc.tile_pool(name="d", bufs=1))
        x_pool = ctx.enter_context(tc.tile_pool(name="x", bufs=3))
        w_pool = ctx.enter_context(tc.tile_pool(name="w", bufs=3))
        t_pool = ctx.enter_context(tc.tile_pool(name="t", bufs=KT))
        z_pool = ctx.enter_context(tc.tile_pool(name="z", bufs=3))
        ps1 = ctx.enter_context(tc.tile_pool(name="ps1", bufs=4, space="PSUM"))
        ps2 = ctx.enter_context(tc.tile_pool(name="ps2", bufs=4, space="PSUM"))

        dh_sb = [
            d_pool.tile([P, N], BF16, tag=f"dh{t}", name=f"dh{t}")
            for t in range(KT)
        ]
        dl_sb = [
            d_pool.tile([P, N], BF16, tag=f"dl{t}", name=f"dl{t}")
            for t in range(KT)
        ]
        for fcol in range(NFC):
            for t in range(KT):
                nc.sync.dma_start(
                    dh_sb[t][:, fcol * FC : (fcol + 1) * FC],
                    dh_ext[t * P : (t + 1) * P, fcol * FC : (fcol + 1) * FC],
                )
                nc.sync.dma_start(
                    dl_sb[t][:, fcol * FC : (fcol + 1) * FC],
                    dl_ext[t * P : (t + 1) * P, fcol * FC : (fcol + 1) * FC],
                )

        for fc in range(NFC):
            t_tiles = []
            for cb in range(KT):
                xht = x_pool.tile([P, N], BF16, tag="xh", name="xht")
                xlt = x_pool.tile([P, N], BF16, tag="xl", name="xlt")
                for t_, ext in ((xht, xh_ext), (xlt, xl_ext)):
                    nc.sync.dma_start(
                        t_[:].rearrange("p (t m) -> p t m", t=KT),
                        ext[:, cb * P : (cb + 1) * P].rearrange(
                            "(t p) m -> p t m", p=P
                        ),
                    )
                pt = ps1.tile([P, FC], F32, tag="ps1", name="pt")
                nmm = 3 * KT
                i = 0
                for rt in range(KT):
                    dh = dh_sb[rt][:, fc * FC : (fc + 1) * FC]
                    dl = dl_sb[rt][:, fc * FC : (fc + 1) * FC]
                    xh = xht[:, rt * P : (rt + 1) * P]
                    xl = xlt[:, rt * P : (rt + 1) * P]
                    for l_, r_ in ((xh, dh), (xh, dl), (xl, dh)):
                        nc.tensor.matmul(
                            pt[:], lhsT=l_, rhs=r_,
                            start=(i == 0), stop=(i == nmm - 1),
                        )
                        i += 1
                # split T on device: th = bf16(T), tl = bf16(T - th)
                th = t_pool.tile([P, FC], BF16, tag="th", name="th")
                tl = t_pool.tile([P, FC], BF16, tag="tl", name="tl")
                tmp = w_pool.tile([P, FC], F32, tag="tmp", name="tmp")
                nc.vector.tensor_copy(th[:], pt[:])
                nc.scalar.copy(tmp[:], th[:])
                nc.vector.tensor_sub(tmp[:], pt[:], tmp[:])
                nc.vector.tensor_copy(tl[:], tmp[:])
                t_tiles.append((th, tl))

            for fb in range(FC // P):
                for g in range(NFC):
                    pz = ps2.tile([P, FC], F32, tag="ps2", name="pz")
                    nmm = 3 * KT
                    i = 0
                    for ct in range(KT):
                        th, tl = t_tiles[ct]
                        dh = dh_sb[ct][:, g * FC : (g + 1) * FC]
                        dl = dl_sb[ct][:, g * FC : (g + 1) * FC]
                        thb = th[:, fb * P : (fb + 1) * P]
                        tlb = tl[:, fb * P : (fb + 1) * P]
                        for l_, r_ in ((thb, dh), (thb, dl), (tlb, dh)):
                            nc.tensor.matmul(
                                pz[:], lhsT=l_, rhs=r_,
                                start=(i == 0), stop=(i == nmm - 1),
                            )
                            i += 1
                    zt = z_pool.tile([P, FC], F32, tag="z", name="zt")
                    nc.vector.tensor_copy(zt[:], pz[:])
                    row0 = (fc * (FC // P) + fb) * P
                    nc.sync.dma_start(
                        z_ext[row0 : row0 + P, g * FC : (g + 1) * FC], zt[:]
                    )

    nc.finalize()
    return nc


_PROGRAM_CACHE: dict = {}


_BUILDERS = {
    "f32r": _build_f32r,
    "bfly": _build_bfly,
    "bfly16": _build_bfly16,
    "lvl2": _build_lvl2,
    "split": _build_split,
}


def _mu_perm() -> np.ndarray:
    """perm[128*s + p] = original c index stored at (block s, partition p).
    Blocks 0-3 ascending, 4-7 descending (mirror about 1023), 8-15 mirror of
    0-7 about 2047 -> both pass-2 fold levels are partition-aligned."""
    mu = np.empty((KT, P), np.int64)
    for t in range(4):
        mu[t] = 128 * t + np.arange(P)
        mu[4 + t] = 1023 - 128 * t - np.arange(P)
    for t in range(8):
        mu[8 + t] = 2047 - mu[t]
    return mu.reshape(-1)


def _lvl2_in_maps(x: np.ndarray):
    import ml_dtypes

    Qm = FC
    MU = _mu_perm()
    n5 = np.arange(Qm, dtype=np.float64)
    phi = np.pi * (2 * n5 + 1) / (4 * H)
    cphi, sphi = np.cos(phi), np.sin(phi)
    sgn = (-1.0) ** n5

    def mat2(M):
        n = np.arange(M, dtype=np.float64)[:, None]
        k = np.arange(M, dtype=np.float64)[None, :]
        return np.cos(np.pi * (2 * n + 1) * k / (2 * M))

    def mat4(M):
        n = np.arange(M, dtype=np.float64)[:, None]
        k = np.arange(M, dtype=np.float64)[None, :]
        return np.cos(np.pi * (2 * n + 1) * (2 * k + 1) / (4 * M))

    d2 = mat2(Qm).astype(ml_dtypes.bfloat16)
    d4 = mat4(Qm).astype(ml_dtypes.bfloat16)
    d2r = d2[:, ::-1].copy()
    # pass-2 tan-rotation: av2' = d1 + tan*d2, bv2' = d2 - tan*d1; the cos
    # (and sign) row-scales are folded into the cd2/scd2r rhs matrices.
    cd2 = (cphi[:, None] * mat2(Qm)).astype(ml_dtypes.bfloat16)
    scd2r = ((sgn * cphi)[:, None] * mat2(Qm)[:, ::-1]).astype(
        ml_dtypes.bfloat16
    )
    rot = np.empty((P, 4), np.float32)
    for t in range(4):
        c2 = (128 * t + np.arange(P)).astype(np.float64)
        ph = np.pi * (2 * c2 + 1) / (4 * H)
        rot[:, t] = np.tan(ph)

    maps = []
    for i in range(B):
        xi = np.asarray(x[i], dtype=np.float32)
        xt, xb = xi[:H], xi[N - 1 : H - 1 : -1]
        u, v = xt + xb, xt - xb
        uu = u[:Qm] + u[H - 1 : Qm - 1 : -1]
        uv = u[:Qm] - u[H - 1 : Qm - 1 : -1]
        vt, vb = v[:Qm], v[H - 1 : Qm - 1 : -1]
        av = vt * cphi[:, None] + vb * sphi[:, None]
        bv = sgn[:, None] * (-vt * sphi[:, None] + vb * cphi[:, None])
        m = {
            "uu": uu[:, MU].astype(ml_dtypes.bfloat16),
            "uv": uv[:, MU].astype(ml_dtypes.bfloat16),
            "av": av[:, MU].astype(ml_dtypes.bfloat16),
            "bv": bv[:, MU].astype(ml_dtypes.bfloat16),
            "d2": d2, "d4": d4, "d2r": d2r, "cd2": cd2,
            "scd2r": scd2r, "rot": rot,
        }
        maps.append(m)
    return maps


def _get_program(mode: str) -> bass.Bass:
    if mode not in _PROGRAM_CACHE:
        _PROGRAM_CACHE[mode] = _BUILDERS[mode]()
    return _PROGRAM_CACHE[mode]


def _make_in_maps(x: np.ndarray, mode: str):
    if mode == "lvl2":
        return _lvl2_in_maps(x)
    if mode == "f32r":
        dr = _round_f32r(_dct_matrix_d())
        return [{"x": _round_f32r(x[i]), "d": dr} for i in range(B)]
    if mode == "bfly16":
        import ml_dtypes

        n2 = np.arange(H, dtype=np.float64)[:, None]
        j2 = np.arange(H, dtype=np.float64)[None, :]
        de = np.cos(np.pi * (2 * n2 + 1) * j2 / N).astype(ml_dtypes.bfloat16)
        do = np.cos(np.pi * (2 * n2 + 1) * (2 * j2 + 1) / (2 * N)).astype(
            ml_dtypes.bfloat16
        )
        maps = []
        for i in range(B):
            xi = np.asarray(x[i], dtype=np.float32)
            xr = xi[::-1]
            u = (xi[:H] + xr[:H]).astype(ml_dtypes.bfloat16)
            v = (xi[:H] - xr[:H]).astype(ml_dtypes.bfloat16)
            # Column-reverse blocks 8..15 so pass-1 bottom-half outputs come
            # out partition-mirrored (see _build_bfly16 pass-2 fold).
            for w in (u, v):
                blk = w[:, H:].reshape(H, KT2, P)
                w[:, H:] = blk[:, :, ::-1].reshape(H, H)
            maps.append({"u": u, "v": v, "de": de, "do": do})
        return maps
    if mode == "bfly":
        n2 = np.arange(H, dtype=np.float64)[:, None]
        j2 = np.arange(H, dtype=np.float64)[None, :]
        de = _round_f32r(np.cos(np.pi * (2 * n2 + 1) * j2 / N).astype(np.float32))
        do = _round_f32r(
            np.cos(np.pi * (2 * n2 + 1) * (2 * j2 + 1) / (2 * N)).astype(
                np.float32
            )
        )
        maps = []
        for i in range(B):
            xi = np.asarray(x[i], dtype=np.float32)
            xr = xi[::-1]
            u = _round_f32r(xi[:H] + xr[:H])
            v = _round_f32r(xi[:H] - xr[:H])
            # Column-reverse blocks 8..15: pass-1 output partitions for those
            # blocks then come out mirror-ordered, which makes the pass-2
            # bottom-half fold a plain ascending read (see _build_bfly).
            for w in (u, v):
                blk = w[:, H:].reshape(H, KT2, P)
                w[:, H:] = blk[:, :, ::-1].reshape(H, H)
            maps.append({"u": u, "v": v, "de": de, "do": do})
        return maps
    dh, dl = _split_bf16(_dct_matrix_d())
    maps = []
    for i in range(B):
        xh, xl = _split_bf16(np.ascontiguousarray(x[i], dtype=np.float32))
        maps.append({"xh": xh, "xl": xl, "dh": dh, "dl": dl})
    return maps


def kernel(x: np.ndarray) -> np.ndarray:
    x = np.asarray(x)
    assert x.shape == (B, N, N), x.shape
    nc = _get_program(MODE)
    in_maps = _make_in_maps(x, MODE)
    res = run_bass_kernel_spmd(nc, in_maps, list(range(B)))
    out = np.stack([res.results[i]["z"] for i in range(B)], axis=0)
    return out.astype(np.float32, copy=False)



# revision 20
# speedup vs baseline: 1.6877x; 1.0366x over previous
"""Trainium2 Bass kernel: batched 2D DCT-II (unnormalized), x: (8, 2048, 2048) f32.

Math: per image X, the unnormalized 2D DCT-II is Z = C @ X @ C^T with
C[k,n] = cos(pi*(2n+1)*k/(2N)).  Let D = C^T.  Using the PE's
out = lhsT.T @ rhs semantics with the *data* as the stationary operand and D as
the moving operand, the two passes chain with no transposes:
    pass 1:  T = X^T @ D      (lhsT = X tiles,  rhs = D)   -> T[c, f]
    pass 2:  Z = T^T @ D      (lhsT = T tiles,  rhs = D)   -> Z = D^T X D = C X C^T

Sharding: batch dim 8 -> one image per NeuronCore (data parallel, no comms).

Dtype modes:
  "f32r"  - single fp32r matmul per term (full PE rate; TF32-like 11-bit
            mantissa operand rounding; ~2e-4 relative-to-absmax error).
  "split" - hi/lo bf16 decomposition, 3 matmuls per term (~5e-6 error, 3x cost).
"""

import numpy as np
from contextlib import ExitStack

import concourse.bass as bass
import concourse.bacc as bacc
import concourse.tile as tile
from concourse import mybir
from concourse.bass_utils import run_bass_kernel_spmd

F32 = mybir.dt.float32
F32R = mybir.dt.float32r
BF16 = mybir.dt.bfloat16

import os

MODE = os.environ.get("DCT_MODE", "lvl2")  # "lvl2", "bfly16", "bfly", "f32r", "split"

B = 8          # batch == n_cores
N = 2048       # image is N x N
P = 128        # partitions
KT = N // P    # 16 k-tiles along any contraction
FC = 512       # chunk width (pass-1 f-chunk, pass-2 g-chunk, PSUM bank)
NFC = N // FC  # 4 chunks
H = N // 2     # butterfly half size
KT2 = H // P   # 8 k-tiles at half contraction


def _round_f32r(a: np.ndarray) -> np.ndarray:
    """fp32r = round-to-nearest, 11 explicit mantissa bits (drop low 12)."""
    b = np.ascontiguousarray(a, dtype=np.float32).view(np.uint32)
    r = ((b + np.uint32(0x800)) & np.uint32(0xFFFFF000)).view(np.float32)
    return r


def _split_bf16(a: np.ndarray):
    import ml_dtypes

    hi = a.astype(ml_dtypes.bfloat16)
    lo = (a - hi.astype(np.float32)).astype(ml_dtypes.bfloat16)
    return hi, lo


def _dct_matrix_d() -> np.ndarray:
    # D[n, k] = cos(pi * (2n+1) * k / (2N)), exact in float64
    n = np.arange(N, dtype=np.float64)[:, None]
    k = np.arange(N, dtype=np.float64)[None, :]
    d = np.cos(np.pi * (2.0 * n + 1.0) * k / (2.0 * N))
    return d.astype(np.float32)


def _build_f32r() -> bass.Bass:
    """fp32r two-pass DCT with the intermediate T round-tripped via DRAM.

    Pass 1 streams X once (one column-block per chain, all 4 f-chunks while
    the block is resident).  T chunks are written back to a DRAM scratch and
    re-streamed as pass-2 stationary tiles.  D stays resident in SBUF.
    """
    nc = bacc.Bacc(None, target_bir_lowering=False)
    x_ext = nc.declare_dram_parameter("x", [N, N], F32R, isOutput=False)
    d_ext = nc.declare_dram_parameter("d", [N, N], F32R, isOutput=False)
    z_ext = nc.declare_dram_parameter("z", [N, N], F32, isOutput=True)

    with ExitStack() as ctx:
        tc = ctx.enter_context(tile.TileContext(nc))
        d_pool = ctx.enter_context(tc.tile_pool(name="d", bufs=1))
        x_pool = ctx.enter_context(tc.tile_pool(name="x", bufs=3))
        t_pool = ctx.enter_context(tc.tile_pool(name="t", bufs=6))
        z_pool = ctx.enter_context(tc.tile_pool(name="z", bufs=3))
        dram = ctx.enter_context(tc.tile_pool(name="dram", bufs=1, space="DRAM"))
        ps1 = ctx.enter_context(tc.tile_pool(name="ps1", bufs=4, space="PSUM"))
        ps2 = ctx.enter_context(tc.tile_pool(name="ps2", bufs=4, space="PSUM"))

        t_dram = dram.tile([N, N], F32R, name="t_dram")

        # First column-block of X loads before D so pass 1 starts early.
        d_sb = [
            d_pool.tile([P, N], F32R, tag=f"d{t}", name=f"d{t}") for t in range(KT)
        ]

        def load_x(cb):
            xt = x_pool.tile([P, N], F32R, tag="x", name="xt")
            nc.sync.dma_start(
                xt[:].rearrange("p (t m) -> p t m", t=KT),
                x_ext[:, cb * P : (cb + 1) * P].rearrange("(t p) m -> p t m", p=P),
            )
            return xt

        x0 = load_x(0)
        # D f-chunk 0 for all 16 row-tiles (pass-1 chain 0 needs only these)
        for fcol in range(NFC):
            for t in range(KT):
                nc.sync.dma_start(
                    d_sb[t][:, fcol * FC : (fcol + 1) * FC],
                    d_ext[t * P : (t + 1) * P, fcol * FC : (fcol + 1) * FC],
                )
            if fcol == 0:
                # remaining D chunks stream behind pass-1 compute
                pass

        # ---- pass 1: per column-block cb, all f-chunks: T[cb,:] = (X^T D)[cb,:]
        for cb in range(KT):
            xt = x0 if cb == 0 else load_x(cb)
            for fc in range(NFC):
                pt = ps1.tile([P, FC], F32, tag="ps1", name="pt")
                for rt in range(KT):
                    nc.tensor.matmul(
                        pt[:],
                        lhsT=xt[:, rt * P : (rt + 1) * P],
                        rhs=d_sb[rt][:, fc * FC : (fc + 1) * FC],
                        start=(rt == 0),
                        stop=(rt == KT - 1),
                    )
                tt = t_pool.tile([P, FC], F32R, tag="t", name="tt")
                nc.vector.tensor_copy(tt[:], pt[:])
                nc.scalar.dma_start(
                    t_dram[cb * P : (cb + 1) * P, fc * FC : (fc + 1) * FC], tt[:]
                )

        # ---- pass 2: per f-block fb: Z[fb,:] = (T^T D)[fb,:]
        for fb in range(KT):
            tf = x_pool.tile([P, N], F32R, tag="x", name="tf")
            nc.sync.dma_start(
                tf[:].rearrange("p (t m) -> p t m", t=KT),
                t_dram[:, fb * P : (fb + 1) * P].rearrange("(t p) m -> p t m", p=P),
            )
            for g in range(NFC):
                pz = ps2.tile([P, FC], F32, tag="ps2", name="pz")
                for ct in range(KT):
                    nc.tensor.matmul(
                        pz[:],
                        lhsT=tf[:, ct * P : (ct + 1) * P],
                        rhs=d_sb[ct][:, g * FC : (g + 1) * FC],
                        start=(ct == 0),
                        stop=(ct == KT - 1),
                    )
                zt = z_pool.tile([P, FC], F32, tag="z", name="zt")
                nc.vector.tensor_copy(zt[:], pz[:])
                nc.scalar.dma_start(
                    z_ext[fb * P : (fb + 1) * P, g * FC : (g + 1) * FC], zt[:]
                )

    nc.finalize()
    return nc


def _build_bfly() -> bass.Bass:
    """Radix-2 even/odd DCT factorization in fp32r: each 1D DCT-II of size N
    becomes two size-N/2 cosine transforms of the folded sequences
    u = x_top + reverse(x_bot), v = x_top - reverse(x_bot):
        y[2j]   = sum_{n<H} u[n] De[n, j],   De[n,j] = cos(pi (2n+1) j / N)
        y[2j+1] = sum_{n<H} v[n] Do[n, j],   Do[n,j] = cos(pi (2n+1)(2j+1) / 2N)
    Halves the matmul work per pass.  Pass-1 folding is done on the host
    (u/v uploaded); pass-2 folding of the intermediate T is done by DVE with a
    reversed-row DMA load.  Outputs are de-interleaved on chip (strided DVE
    writes) + stride-2-row DMA stores, so all DRAM traffic stays contiguous
    per partition.
    """
    nc = bacc.Bacc(None, target_bir_lowering=False)
    u_ext = nc.declare_dram_parameter("u", [H, N], F32R, isOutput=False)
    v_ext = nc.declare_dram_parameter("v", [H, N], F32R, isOutput=False)
    de_ext = nc.declare_dram_parameter("de", [H, H], F32R, isOutput=False)
    do_ext = nc.declare_dram_parameter("do", [H, H], F32R, isOutput=False)
    z_ext = nc.declare_dram_parameter("z", [N, N], F32, isOutput=True)

    with ExitStack() as ctx:
        tc = ctx.enter_context(tile.TileContext(nc))
        d_pool = ctx.enter_context(tc.tile_pool(name="d", bufs=1))
        x_pool = ctx.enter_context(tc.tile_pool(name="x", bufs=4))
        t_pool = ctx.enter_context(tc.tile_pool(name="t", bufs=4))
        b_pool = ctx.enter_context(tc.tile_pool(name="b", bufs=4))
        z_pool = ctx.enter_context(tc.tile_pool(name="z", bufs=3))
        dram = ctx.enter_context(tc.tile_pool(name="dram", bufs=1, space="DRAM"))
        # PSUM: pass-1 accumulators 2x1 bank; pass-2 output chains share one
        # 6-slot pool (6 banks) so slow de-interleave copies never stall the
        # next chain. 8 banks total (no reversal matmuls anymore).
        ps1 = ctx.enter_context(tc.tile_pool(name="ps1", bufs=2, space="PSUM"))
        ps2 = ctx.enter_context(tc.tile_pool(name="ps2", bufs=6, space="PSUM"))

        # T in blocked layout: cols [0,H) = even outputs, [H,2H) = odd
        t_dram = dram.tile([N, N], F32R, name="t_dram")

        de_sb = [
            d_pool.tile([P, H], F32R, tag=f"de{t}", name=f"de{t}")
            for t in range(KT2)
        ]
        do_sb = [
            d_pool.tile([P, H], F32R, tag=f"do{t}", name=f"do{t}")
            for t in range(KT2)
        ]

        def load_block(ext, cb, tag):
            w = x_pool.tile([P, H], F32R, tag=tag, name="w_" + tag)
            nc.sync.dma_start(
                w[:].rearrange("p (t m) -> p t m", t=KT2),
                ext[:, cb * P : (cb + 1) * P].rearrange("(t p) m -> p t m", p=P),
            )
            return w

        # loads in exact first-consumption order: u0, de jc0, de jc1, v0,
        # do jc0, do jc1; the pass-2 reversal matrix r last.
        u0 = load_block(u_ext, 0, "u")
        for jc in range(2):
            for t in range(KT2):
                nc.sync.dma_start(
                    de_sb[t][:, jc * FC : (jc + 1) * FC],
                    de_ext[t * P : (t + 1) * P, jc * FC : (jc + 1) * FC],
                )
        v0 = load_block(v_ext, 0, "v")
        for jc in range(2):
            for t in range(KT2):
                nc.sync.dma_start(
                    do_sb[t][:, jc * FC : (jc + 1) * FC],
                    do_ext[t * P : (t + 1) * P, jc * FC : (jc + 1) * FC],
                )

        # ---- pass 1: T_blk[cb, :] ----
        for cb in range(KT):
            ut = u0 if cb == 0 else load_block(u_ext, cb, "u")
            vt = v0 if cb == 0 else load_block(v_ext, cb, "v")
            for half, (wt, dsb) in enumerate(((ut, de_sb), (vt, do_sb))):
                for jc in range(2):
                    pt = ps1.tile([P, FC], F32, tag="acc", name="pt")
                    for rt in range(KT2):
                        nc.tensor.matmul(
                            pt[:],
                            lhsT=wt[:, rt * P : (rt + 1) * P],
                            rhs=dsb[rt][:, jc * FC : (jc + 1) * FC],
                            start=(rt == 0),
                            stop=(rt == KT2 - 1),
                        )
                    tt = t_pool.tile([P, FC], F32R, tag="t", name="tt")
                    nc.vector.tensor_copy(tt[:], pt[:])
                    col0 = half * H + jc * FC
                    # Bottom-half blocks (cb>=8) arrive partition-reversed
                    # (host reversed their lhsT columns) and are stored
                    # mirror-ordered: row 1024+k holds T[2047-k].  Then the
                    # pass-2 fold reads both halves with plain ascending loads.
                    row0 = cb * P if cb < KT2 else (23 * P - cb * P)
                    nc.scalar.dma_start(
                        t_dram[row0 : row0 + P, col0 : col0 + FC], tt[:]
                    )

        # ---- pass 2: fold T over rows, transform, de-interleave out ----
        # bot_rev[c', f] = T[2047-c', f]: partition reversal via one PE matmul
        # with the reversal permutation R (out[m,n] = bot[127-m, n]); the
        # tile-order flip (ct -> 7-ct) via a reversed free-dim view in the add.
        # Software-pipelined: loads run 3 blocks ahead, reversal matmul + DVE
        # fold 2 ahead, so block fb's chains never wait on its fold.
        folded: dict = {}

        def p2_load(fb):
            top = b_pool.tile([P, H], F32R, tag="top", name="top")
            nc.sync.dma_start(
                top[:].rearrange("p (t m) -> p t m", t=KT2),
                t_dram[0:H, fb * P : (fb + 1) * P].rearrange(
                    "(t p) m -> p t m", p=P
                ),
            )
            bot = b_pool.tile([P, H], F32R, tag="bot", name="bot")
            nc.sync.dma_start(
                bot[:].rearrange("p (t m) -> p t m", t=KT2),
                t_dram[H:N, fb * P : (fb + 1) * P].rearrange(
                    "(t p) m -> p t m", p=P
                ),
            )
            folded[fb] = (top, bot)

        def p2_fold(fb):
            # mirror-ordered bottom storage makes the fold a plain 2D add/sub
            top, bot = folded[fb]
            u2 = b_pool.tile([P, H], F32R, tag="u2", name="u2")
            nc.vector.tensor_add(u2[:], top[:], bot[:])
            v2 = b_pool.tile([P, H], F32R, tag="v2", name="v2")
            nc.vector.tensor_sub(v2[:], top[:], bot[:])
            folded[fb] = (u2, v2)

        p2_load(0)
        p2_load(1)
        p2_fold(0)
        p2_load(2)
        p2_fold(1)
        for fb in range(KT):
            u2, v2 = folded.pop(fb)
            # f_blk block fb -> actual Z rows (de-interleave rows via stride 2)
            if fb < KT2:
                row0 = 2 * fb * P
                row_stop = row0 + 2 * P
            else:
                row0 = 2 * (fb - KT2) * P + 1
                row_stop = row0 + 2 * P - 1
            for jc in range(2):
                pe_ = ps2.tile([P, FC], F32, tag="zacc", name="pe_")
                for ct in range(KT2):
                    nc.tensor.matmul(
                        pe_[:],
                        lhsT=u2[:, ct * P : (ct + 1) * P],
                        rhs=de_sb[ct][:, jc * FC : (jc + 1) * FC],
                        start=(ct == 0),
                        stop=(ct == KT2 - 1),
                    )
                po_ = ps2.tile([P, FC], F32, tag="zacc", name="po_")
                for ct in range(KT2):
                    nc.tensor.matmul(
                        po_[:],
                        lhsT=v2[:, ct * P : (ct + 1) * P],
                        rhs=do_sb[ct][:, jc * FC : (jc + 1) * FC],
                        start=(ct == 0),
                        stop=(ct == KT2 - 1),
                    )
                zt = z_pool.tile([P, 2 * FC], F32, tag="z", name="zt")
                nc.scalar.copy(zt[:, 0 : 2 * FC : 2], pe_[:])
                nc.vector.tensor_copy(zt[:, 1 : 2 * FC : 2], po_[:])
                nc.scalar.dma_start(
                    z_ext[row0:row_stop:2, jc * 2 * FC : (jc + 1) * 2 * FC],
                    zt[:],
                )
            if fb + 3 < KT:
                p2_load(fb + 3)
            if fb + 2 < KT:
                p2_fold(fb + 2)

    nc.finalize()
    return nc


def _build_bfly16() -> bass.Bass:
    """bf16 radix-2 butterfly DCT with the intermediate T kept entirely in
    SBUF (no DRAM roundtrip).  Same math/layout as _build_bfly: pass-1 folding
    host-side (u/v uploaded, bottom column-blocks mirrored), blocked T
    [even|odd], pass-2 fold via partition-aligned mirror blocks, outputs
    de-interleaved on chip + stride-2 row DMA stores.  bf16 operands double
    the PE rate vs fp32r and halve SBUF/DMA footprint; PSUM accumulates f32.
    """
    nc = bacc.Bacc(None, target_bir_lowering=False)
    u_ext = nc.declare_dram_parameter("u", [H, N], BF16, isOutput=False)
    v_ext = nc.declare_dram_parameter("v", [H, N], BF16, isOutput=False)
    de_ext = nc.declare_dram_parameter("de", [H, H], BF16, isOutput=False)
    do_ext = nc.declare_dram_parameter("do", [H, H], BF16, isOutput=False)
    z_ext = nc.declare_dram_parameter("z", [N, N], F32, isOutput=True)

    with ExitStack() as ctx:
        tc = ctx.enter_context(tile.TileContext(nc))
        d_pool = ctx.enter_context(tc.tile_pool(name="d", bufs=1))
        x_pool = ctx.enter_context(tc.tile_pool(name="x", bufs=4))
        t_pool = ctx.enter_context(tc.tile_pool(name="t", bufs=1))
        f_pool = ctx.enter_context(tc.tile_pool(name="f", bufs=1))
        z_pool = ctx.enter_context(tc.tile_pool(name="z", bufs=3))
        ps1 = ctx.enter_context(tc.tile_pool(name="ps1", bufs=2, space="PSUM"))
        ps2 = ctx.enter_context(tc.tile_pool(name="ps2", bufs=6, space="PSUM"))

        de_sb = [
            d_pool.tile([P, H], BF16, tag=f"de{t}", name=f"de{t}")
            for t in range(KT2)
        ]
        do_sb = [
            d_pool.tile([P, H], BF16, tag=f"do{t}", name=f"do{t}")
            for t in range(KT2)
        ]
        # T resident in SBUF: storage block s holds pass-1 output of
        # cb = s (s<8) or cb = 23-s (s>=8, partition-mirrored rows).
        t_sb = [
            t_pool.tile([P, N], BF16, tag=f"t{s}", name=f"t{s}")
            for s in range(KT)
        ]

        def load_block(ext, cb, tag):
            w = x_pool.tile([P, H], BF16, tag=tag, name="w_" + tag)
            nc.sync.dma_start(
                w[:].rearrange("p (t m) -> p t m", t=KT2),
                ext[:, cb * P : (cb + 1) * P].rearrange("(t p) m -> p t m", p=P),
            )
            return w

        u0 = load_block(u_ext, 0, "u")
        for jc in range(2):
            for t in range(KT2):
                nc.sync.dma_start(
                    de_sb[t][:, jc * FC : (jc + 1) * FC],
                    de_ext[t * P : (t + 1) * P, jc * FC : (jc + 1) * FC],
                )
        v0 = load_block(v_ext, 0, "v")
        for jc in range(2):
            for t in range(KT2):
                nc.sync.dma_start(
                    do_sb[t][:, jc * FC : (jc + 1) * FC],
                    do_ext[t * P : (t + 1) * P, jc * FC : (jc + 1) * FC],
                )

        # ---- pass 1: T_blk[cb, :] straight into SBUF ----
        for cb in range(KT):
            ut = u0 if cb == 0 else load_block(u_ext, cb, "u")
            vt = v0 if cb == 0 else load_block(v_ext, cb, "v")
            s = cb if cb < KT2 else 23 - cb
            for half, (wt, dsb) in enumerate(((ut, de_sb), (vt, do_sb))):
                for jc in range(2):
                    pt = ps1.tile([P, FC], F32, tag="acc", name="pt")
                    for rt in range(KT2):
                        nc.tensor.matmul(
                            pt[:],
                            lhsT=wt[:, rt * P : (rt + 1) * P],
                            rhs=dsb[rt][:, jc * FC : (jc + 1) * FC],
                            start=(rt == 0),
                            stop=(rt == KT2 - 1),
                        )
                    col0 = half * H + jc * FC
                    nc.vector.tensor_copy(
                        t_sb[s][:, col0 : col0 + FC], pt[:]
                    )

        # ---- pass 2: fold T in SBUF, transform, de-interleave out ----
        u2 = [
            f_pool.tile([P, N], BF16, tag=f"u2_{t}", name=f"u2_{t}")
            for t in range(KT2)
        ]
        v2 = [
            f_pool.tile([P, N], BF16, tag=f"v2_{t}", name=f"v2_{t}")
            for t in range(KT2)
        ]
        # fold order t=7..0: pair (t, 8+t) is ready once cb=15-t finished,
        # so later folds wait on earlier pass-1 chains.
        for t in range(KT2 - 1, -1, -1):
            nc.vector.tensor_add(u2[t][:], t_sb[t][:], t_sb[KT2 + t][:])
            nc.vector.tensor_sub(v2[t][:], t_sb[t][:], t_sb[KT2 + t][:])

        for fb in range(KT):
            if fb < KT2:
                row0 = 2 * fb * P
                row_stop = row0 + 2 * P
            else:
                row0 = 2 * (fb - KT2) * P + 1
                row_stop = row0 + 2 * P - 1
            for jc in range(2):
                pe_ = ps2.tile([P, FC], F32, tag="zacc", name="pe_")
                for ct in range(KT2 - 1, -1, -1):
                    nc.tensor.matmul(
                        pe_[:],
                        lhsT=u2[ct][:, fb * P : (fb + 1) * P],
                        rhs=de_sb[ct][:, jc * FC : (jc + 1) * FC],
                        start=(ct == KT2 - 1),
                        stop=(ct == 0),
                    )
                po_ = ps2.tile([P, FC], F32, tag="zacc", name="po_")
                for ct in range(KT2 - 1, -1, -1):
                    nc.tensor.matmul(
                        po_[:],
                        lhsT=v2[ct][:, fb * P : (fb + 1) * P],
                        rhs=do_sb[ct][:, jc * FC : (jc + 1) * FC],
                        start=(ct == KT2 - 1),
                        stop=(ct == 0),
                    )
                zt = z_pool.tile([P, 2 * FC], F32, tag="z", name="zt")
                nc.scalar.copy(zt[:, 0 : 2 * FC : 2], pe_[:])
                nc.vector.tensor_copy(zt[:, 1 : 2 * FC : 2], po_[:])
                nc.scalar.dma_start(
                    z_ext[row0:row_stop:2, jc * 2 * FC : (jc + 1) * 2 * FC],
                    zt[:],
                )

    nc.finalize()
    return nc


def _build_lvl2() -> bass.Bass:
    """Level-2 pole-free butterfly DCT (bf16, T in SBUF).

    1D DCT-II_2048 factored twice:
      fold1: u = xt + xb_rev (DCT-II_1024), v = xt - xb_rev (DCT-IV_1024)
      u: fold2 -> uu (DCT-II_512), uv (DCT-IV_512)
      v (DCT-IV_1024, stable rotation form): av/bv Givens-rotated pairs,
         both through DCT-II_512 (bv with column-reversed matrix), then a
         post-butterfly of adjacent outputs: yodd[2j] = C[j] + S[j],
         yodd[2j+1] = C[j+1] - S[j+1], S[j] = SBV[Q-j].
    Pass-1 folds/rotations on host; pass-2 folds/rotations on device from
    SBUF-resident blocked T.  mu column permutation makes both pass-2 fold
    levels partition-aligned.  PE work is N^3/4 per pass (half of level-1).
    """
    nc = bacc.Bacc(None, target_bir_lowering=False)
    Qm = FC  # 512
    uu_ext = nc.declare_dram_parameter("uu", [Qm, N], BF16, isOutput=False)
    uv_ext = nc.declare_dram_parameter("uv", [Qm, N], BF16, isOutput=False)
    av_ext = nc.declare_dram_parameter("av", [Qm, N], BF16, isOutput=False)
    bv_ext = nc.declare_dram_parameter("bv", [Qm, N], BF16, isOutput=False)
    d2_ext = nc.declare_dram_parameter("d2", [Qm, Qm], BF16, isOutput=False)
    d4_ext = nc.declare_dram_parameter("d4", [Qm, Qm], BF16, isOutput=False)
    d2r_ext = nc.declare_dram_parameter("d2r", [Qm, Qm], BF16, isOutput=False)
    cd2_ext = nc.declare_dram_parameter("cd2", [Qm, Qm], BF16, isOutput=False)
    scd2r_ext = nc.declare_dram_parameter("scd2r", [Qm, Qm], BF16, isOutput=False)
    rot_ext = nc.declare_dram_parameter("rot", [P, 4], F32, isOutput=False)
    z_ext = nc.declare_dram_parameter("z", [N, N], F32, isOutput=True)

    AOT = __import__("concourse.alu_op_type", fromlist=["AluOpType"]).AluOpType

    with ExitStack() as ctx:
        tc = ctx.enter_context(tile.TileContext(nc))
        d_pool = ctx.enter_context(tc.tile_pool(name="d", bufs=1))
        x_pool = ctx.enter_context(tc.tile_pool(name="x", bufs=3))
        t_pool = ctx.enter_context(tc.tile_pool(name="t", bufs=1))
        f_pool = ctx.enter_context(tc.tile_pool(name="f", bufs=1))
        s_pool = ctx.enter_context(tc.tile_pool(name="s", bufs=1))
        z_pool = ctx.enter_context(tc.tile_pool(name="z", bufs=2))
        ps = ctx.enter_context(tc.tile_pool(name="ps", bufs=2, space="PSUM"))

        d2_sb = [d_pool.tile([P, Qm], BF16, tag=f"d2_{k}", name=f"d2_{k}") for k in range(4)]
        d4_sb = [d_pool.tile([P, Qm], BF16, tag=f"d4_{k}", name=f"d4_{k}") for k in range(4)]
        d2r_sb = [d_pool.tile([P, Qm], BF16, tag=f"d2r_{k}", name=f"d2r_{k}") for k in range(4)]
        cd2_sb = [d_pool.tile([P, Qm], BF16, tag=f"cd2_{k}", name=f"cd2_{k}") for k in range(4)]
        scd2r_sb = [d_pool.tile([P, Qm], BF16, tag=f"scd2r_{k}", name=f"scd2r_{k}") for k in range(4)]
        rot_sb = d_pool.tile([P, 4], F32, tag="rot", name="rot")
        t_sb = [t_pool.tile([P, N], BF16, tag=f"t{s}", name=f"t{s}") for s in range(KT)]
        uu2 = [f_pool.tile([P, N], BF16, tag=f"uu2_{t}", name=f"uu2_{t}") for t in range(4)]
        uv2 = [f_pool.tile([P, N], BF16, tag=f"uv2_{t}", name=f"uv2_{t}") for t in range(4)]
        av2 = [f_pool.tile([P, N], BF16, tag=f"av2_{t}", name=f"av2_{t}") for t in range(4)]
        bv2 = [f_pool.tile([P, N], BF16, tag=f"bv2_{t}", name=f"bv2_{t}") for t in range(4)]

        def load_w(ext, cb, tag):
            w = x_pool.tile([P, 4 * P], BF16, tag=tag, name="w_" + tag)
            nc.sync.dma_start(
                w[:].rearrange("p (t m) -> p t m", t=4),
                ext[:, cb * P : (cb + 1) * P].rearrange("(t p) m -> p t m", p=P),
            )
            return w

        def load_d(ext, sb, eng=None):
            eng = eng or nc.sync
            for k in range(4):
                eng.dma_start(sb[k][:], ext[k * P : (k + 1) * P, :])

        w0 = [load_w(uu_ext, 0, "uu")]
        load_d(d2_ext, d2_sb)
        w0.append(load_w(uv_ext, 0, "uv"))
        load_d(d4_ext, d4_sb)
        w0.append(load_w(av_ext, 0, "av"))
        w0.append(load_w(bv_ext, 0, "bv"))
        load_d(d2r_ext, d2r_sb, nc.scalar)
        nc.scalar.dma_start(rot_sb[:], rot_ext[:, :])
        load_d(cd2_ext, cd2_sb, nc.scalar)
        load_d(scd2r_ext, scd2r_sb, nc.scalar)

        def fold(t):
            """pass-2 level-2 fold + tan-rotation for c''-block t (needs
            pass-1 cbs t, 4+t, 8+t, 12+t done).  The cos row-scales of the
            rotation live in the cd2/scd2r rhs matrices."""
            p_ = s_pool.tile([P, N], BF16, tag="fp", name="fp")
            q_ = s_pool.tile([P, N], BF16, tag="fq", name="fq")
            nc.vector.tensor_add(p_[:], t_sb[t][:], t_sb[8 + t][:])
            nc.vector.tensor_add(q_[:], t_sb[4 + t][:], t_sb[12 + t][:])
            nc.vector.tensor_add(uu2[t][:], p_[:], q_[:])
            nc.vector.tensor_sub(uv2[t][:], p_[:], q_[:])
            d1 = s_pool.tile([P, N], BF16, tag="fd1", name="fd1")
            d2_ = s_pool.tile([P, N], BF16, tag="fd2", name="fd2")
            t1 = s_pool.tile([P, N], BF16, tag="ft1", name="ft1")
            t2 = s_pool.tile([P, N], BF16, tag="ft2", name="ft2")
            # column halves so the last fold gates pass-2 minimally
            for h in (slice(0, N // 2), slice(N // 2, N)):
                nc.vector.tensor_sub(d1[:, h], t_sb[t][:, h], t_sb[8 + t][:, h])
                nc.vector.tensor_sub(
                    d2_[:, h], t_sb[4 + t][:, h], t_sb[12 + t][:, h]
                )
                nc.scalar.mul(t1[:, h], d2_[:, h], rot_sb[:, t : t + 1])
                nc.vector.tensor_add(av2[t][:, h], d1[:, h], t1[:, h])
                nc.scalar.mul(t2[:, h], d1[:, h], rot_sb[:, t : t + 1])
                nc.vector.tensor_sub(bv2[t][:, h], d2_[:, h], t2[:, h])

        # ---- pass 1 ----
        cb_order = [t + 4 * i for t in range(4) for i in range(4)]
        for idx, cb in enumerate(cb_order):
            ws = w0 if cb == 0 else [
                load_w(uu_ext, cb, "uu"),
                load_w(uv_ext, cb, "uv"),
                load_w(av_ext, cb, "av"),
                load_w(bv_ext, cb, "bv"),
            ]
            dsbs = (d2_sb, d4_sb, d2_sb, d2r_sb)
            pts = []
            for i, (w, dsb) in enumerate(zip(ws, dsbs)):
                pt = ps.tile([P, Qm], F32, tag=f"c{i}", name=f"pt{i}")
                for k in range(4):
                    nc.tensor.matmul(
                        pt[:],
                        lhsT=w[:, k * P : (k + 1) * P],
                        rhs=dsb[k][:],
                        start=(k == 0),
                        stop=(k == 3),
                    )
                pts.append(pt)
            puu, puv, pc, psv = pts
            tcur = t_sb[cb]
            nc.scalar.copy(tcur[:, 0:Qm], puu[:])
            nc.scalar.copy(tcur[:, Qm : 2 * Qm], puv[:])
            nc.scalar.copy(tcur[:, 2 * Qm : 2 * Qm + 1], pc[:, 0:1])
            sv = s_pool.tile([P, Qm], F32, tag="sv", name="sv")
            nc.scalar.copy(sv[:], psv[:])
            nc.vector.tensor_add(
                tcur[:, 2 * Qm + 1 : 3 * Qm], pc[:, 1:Qm], sv[:, 0 : Qm - 1]
            )
            nc.vector.tensor_sub(
                tcur[:, 3 * Qm : 4 * Qm - 1], pc[:, 1:Qm], sv[:, 0 : Qm - 1]
            )
            nc.scalar.mul(tcur[:, 4 * Qm - 1 : 4 * Qm], sv[:, Qm - 1 : Qm], -1.0)
            if idx % 4 == 3:
                fold(idx // 4)

        # ---- pass 2 ----
        off = (0, 2, 1, 3)
        for fb in range(KT):
            srcs = (uu2, uv2, av2, bv2)
            dsbs = (d2_sb, d4_sb, cd2_sb, scd2r_sb)
            pts = []
            for i, (src, dsb) in enumerate(zip(srcs, dsbs)):
                pt = ps.tile([P, Qm], F32, tag=f"c{i}", name=f"zt{i}")
                for k in range(4):
                    nc.tensor.matmul(
                        pt[:],
                        lhsT=src[k][:, fb * P : (fb + 1) * P],
                        rhs=dsb[k][:],
                        start=(k == 0),
                        stop=(k == 3),
                    )
                pts.append(pt)
            puu, puv, pc, psv = pts
            zt = z_pool.tile([P, N], F32, tag="z", name="zt")
            nc.scalar.copy(zt[:, 0 : N : 4], puu[:])
            nc.vector.tensor_copy(zt[:, 2 : N : 4], puv[:])
            nc.scalar.copy(zt[:, 1:2], pc[:, 0:1])
            sv = s_pool.tile([P, Qm], F32, tag="sv", name="sv")
            nc.scalar.copy(sv[:], psv[:])
            nc.vector.tensor_add(
                zt[:, 5 : N : 4], pc[:, 1:Qm], sv[:, 0 : Qm - 1]
            )
            nc.vector.tensor_sub(
                zt[:, 3 : N - 3 : 4], pc[:, 1:Qm], sv[:, 0 : Qm - 1]
            )
            nc.scalar.mul(zt[:, N - 1 : N], sv[:, Qm - 1 : Qm], -1.0)
            row0 = 4 * P * (fb % 4) + off[fb // 4]
            dma_eng = nc.scalar if fb % 2 == 0 else nc.sync
            dma_eng.dma_start(z_ext[row0 : row0 + 4 * P - 3 : 4, :], zt[:])

    nc.finalize()
    return nc


def _build_split() -> bass.Bass:
    """hi/lo bf16 decomposition: each logical matmul = 3 bf16 matmuls
    (Xh Dh + Xh Dl + Xl Dh), accumulated in the same PSUM chain."""
    nc = bacc.Bacc(None, target_bir_lowering=False)
    xh_ext = nc.declare_dram_parameter("xh", [N, N], BF16, isOutput=False)
    xl_ext = nc.declare_dram_parameter("xl", [N, N], BF16, isOutput=False)
    dh_ext = nc.declare_dram_parameter("dh", [N, N], BF16, isOutput=False)
    dl_ext = nc.declare_dram_parameter("dl", [N, N], BF16, isOutput=False)
    z_ext = nc.declare_dram_parameter("z", [N, N], F32, isOutput=True)

    with ExitStack() as ctx:
        tc = ctx.enter_context(tile.TileContext(nc))
        d_pool = ctx.enter_context(tc.tile_pool(name="d", bufs=1))
        x_pool = ctx.enter_context(tc.tile_pool(name="x", bufs=3))
        w_pool = ctx.enter_context(tc.tile_pool(name="w", bufs=3))
        t_pool = ctx.enter_context(tc.tile_pool(name="t", bufs=KT))
        z_pool = ctx.enter_context(tc.tile_pool(name="z", bufs=3))
        ps1 = ctx.enter_context(tc.tile_pool(name="ps1", bufs=4, space="PSUM"))
        ps2 = ctx.enter_context(tc.tile_pool(name="ps2", bufs=4, space="PSUM"))

        dh_sb = [
            d_pool.tile([P, N], BF16, tag=f"dh{t}", name=f"dh{t}")
            for t in range(KT)
        ]
        dl_sb = [
            d_pool.tile([P, N], BF16, tag=f"dl{t}", name=f"dl{t}")
            for t in range(KT)
        ]
        for fcol in range(NFC):
            for t in range(KT):
                nc.sync.dma_start(
                    dh_sb[t][:, fcol * FC : (fcol + 1) * FC],
                    dh_ext[t * P : (t + 1) * P, fcol * FC : (fcol + 1) * FC],
                )
                nc.sync.dma_start(
                    dl_sb[t][:, fcol * FC : (fcol + 1) * FC],
                    dl_ext[t * P : (t + 1) * P, fcol * FC : (fcol + 1) * FC],
                )

        for fc in range(NFC):
            t_tiles = []
            for cb in range(KT):
                xht = x_pool.tile([P, N], BF16, tag="xh", name="xht")
                xlt = x_pool.tile([P, N], BF16, tag="xl", name="xlt")
                for t_, ext in ((xht, xh_ext), (xlt, xl_ext)):
                    nc.sync.dma_start(
                        t_[:].rearrange("p (t m) -> p t m", t=KT),
                        ext[:, cb * P : (cb + 1) * P].rearrange(
                            "(t p) m -> p t m", p=P
                        ),
                    )
                pt = ps1.tile([P, FC], F32, tag="ps1", name="pt")
                nmm = 3 * KT
                i = 0
                for rt in range(KT):
                    dh = dh_sb[rt][:, fc * FC : (fc + 1) * FC]
                    dl = dl_sb[rt][:, fc * FC : (fc + 1) * FC]
                    xh = xht[:, rt * P : (rt + 1) * P]
                    xl = xlt[:, rt * P : (rt + 1) * P]
                    for l_, r_ in ((xh, dh), (xh, dl), (xl, dh)):
                        nc.tensor.matmul(
                            pt[:], lhsT=l_, rhs=r_,
                            start=(i == 0), stop=(i == nmm - 1),
                        )
                        i += 1
                # split T on device: th = bf16(T), tl = bf16(T - th)
                th = t_pool.tile([P, FC], BF16, tag="th", name="th")
                tl = t_pool.tile([P, FC], BF16, tag="tl", name="tl")
                tmp = w_pool.tile([P, FC], F32, tag="tmp", name="tmp")
                nc.vector.tensor_copy(th[:], pt[:])
                nc.scalar.copy(tmp[:], th[:])
                nc.vector.tensor_sub(tmp[:], pt[:], tmp[:])
                nc.vector.tensor_copy(tl[:], tmp[:])
                t_tiles.append((th, tl))

            for fb in range(FC // P):
                for g in range(NFC):
                    pz = ps2.tile([P, FC], F32, tag="ps2", name="pz")
                    nmm = 3 * KT
                    i = 0
                    for ct in range(KT):
                        th, tl = t_tiles[ct]
                        dh = dh_sb[ct][:, g * FC : (g + 1) * FC]
                        dl = dl_sb[ct][:, g * FC : (g + 1) * FC]
                        thb = th[:, fb * P : (fb + 1) * P]
                        tlb = tl[:, fb * P : (fb + 1) * P]
                        for l_, r_ in ((thb, dh), (thb, dl), (tlb, dh)):
                            nc.tensor.matmul(
                                pz[:], lhsT=l_, rhs=r_,
                                start=(i == 0), stop=(i == nmm - 1),
                            )
                            i += 1
                    zt = z_pool.tile([P, FC], F32, tag="z", name="zt")
                    nc.vector.tensor_copy(zt[:], pz[:])
                    row0 = (fc * (FC // P) + fb) * P
                    nc.sync.dma_start(
                        z_ext[row0 : row0 + P, g * FC : (g + 1) * FC], zt[:]
                    )

    nc.finalize()
    return nc


_PROGRAM_CACHE: dict = {}


_BUILDERS = {
    "f32r": _build_f32r,
    "bfly": _build_bfly,
    "bfly16": _build_bfly16,
    "lvl2": _build_lvl2,
    "split": _build_split,
}


def _mu_perm() -> np.ndarray:
    """perm[128*s + p] = original c index stored at (block s, partition p).
    Blocks 0-3 ascending, 4-7 descending (mirror about 1023), 8-15 mirror of
    0-7 about 2047 -> both pass-2 fold levels are partition-aligned."""
    mu = np.empty((KT, P), np.int64)
    for t in range(4):
        mu[t] = 128 * t + np.arange(P)
        mu[4 + t] = 1023 - 128 * t - np.arange(P)
    for t in range(8):
        mu[8 + t] = 2047 - mu[t]
    return mu.reshape(-1)


def _lvl2_in_maps(x: np.ndarray):
    import ml_dtypes

    Qm = FC
    MU = _mu_perm()
    n5 = np.arange(Qm, dtype=np.float64)
    phi = np.pi * (2 * n5 + 1) / (4 * H)
    cphi, sphi = np.cos(phi), np.sin(phi)
    sgn = (-1.0) ** n5

    def mat2(M):
        n = np.arange(M, dtype=np.float64)[:, None]
        k = np.arange(M, dtype=np.float64)[None, :]
        return np.cos(np.pi * (2 * n + 1) * k / (2 * M))

    def mat4(M):
        n = np.arange(M, dtype=np.float64)[:, None]
        k = np.arange(M, dtype=np.float64)[None, :]
        return np.cos(np.pi * (2 * n + 1) * (2 * k + 1) / (4 * M))

    d2 = mat2(Qm).astype(ml_dtypes.bfloat16)
    d4 = mat4(Qm).astype(ml_dtypes.bfloat16)
    d2r = d2[:, ::-1].copy()
    # pass-2 tan-rotation: av2' = d1 + tan*d2, bv2' = d2 - tan*d1; the cos
    # (and sign) row-scales are folded into the cd2/scd2r rhs matrices.
    cd2 = (cphi[:, None] * mat2(Qm)).astype(ml_dtypes.bfloat16)
    scd2r = ((sgn * cphi)[:, None] * mat2(Qm)[:, ::-1]).astype(
        ml_dtypes.bfloat16
    )
    rot = np.empty((P, 4), np.float32)
    for t in range(4):
        c2 = (128 * t + np.arange(P)).astype(np.float64)
        ph = np.pi * (2 * c2 + 1) / (4 * H)
        rot[:, t] = np.tan(ph)

    maps = []
    for i in range(B):
        xi = np.asarray(x[i], dtype=np.float32)
        xt, xb = xi[:H], xi[N - 1 : H - 1 : -1]
        u, v = xt + xb, xt - xb
        uu = u[:Qm] + u[H - 1 : Qm - 1 : -1]
        uv = u[:Qm] - u[H - 1 : Qm - 1 : -1]
        vt, vb = v[:Qm], v[H - 1 : Qm - 1 : -1]
        av = vt * cphi[:, None] + vb * sphi[:, None]
        bv = sgn[:, None] * (-vt * sphi[:, None] + vb * cphi[:, None])
        m = {
            "uu": uu[:, MU].astype(ml_dtypes.bfloat16),
            "uv": uv[:, MU].astype(ml_dtypes.bfloat16),
            "av": av[:, MU].astype(ml_dtypes.bfloat16),
            "bv": bv[:, MU].astype(ml_dtypes.bfloat16),
            "d2": d2, "d4": d4, "d2r": d2r, "cd2": cd2,
            "scd2r": scd2r, "rot": rot,
        }
        maps.append(m)
    return maps


def _get_program(mode: str) -> bass.Bass:
    if mode not in _PROGRAM_CACHE:
        _PROGRAM_CACHE[mode] = _BUILDERS[mode]()
    return _PROGRAM_CACHE[mode]


def _make_in_maps(x: np.ndarray, mode: str):
    if mode == "lvl2":
        return _lvl2_in_maps(x)
    if mode == "f32r":
        dr = _round_f32r(_dct_matrix_d())
        return [{"x": _round_f32r(x[i]), "d": dr} for i in range(B)]
    if mode == "bfly16":
        import ml_dtypes

        n2 = np.arange(H, dtype=np.float64)[:, None]
        j2 = np.arange(H, dtype=np.float64)[None, :]
        de = np.cos(np.pi * (2 * n2 + 1) * j2 / N).astype(ml_dtypes.bfloat16)
        do = np.cos(np.pi * (2 * n2 + 1) * (2 * j2 + 1) / (2 * N)).astype(
            ml_dtypes.bfloat16
        )
        maps = []
        for i in range(B):
            xi = np.asarray(x[i], dtype=np.float32)
            xr = xi[::-1]
            u = (xi[:H] + xr[:H]).astype(ml_dtypes.bfloat16)
            v = (xi[:H] - xr[:H]).astype(ml_dtypes.bfloat16)
            # Column-reverse blocks 8..15 so pass-1 bottom-half outputs come
            # out partition-mirrored (see _build_bfly16 pass-2 fold).
            for w in (u, v):
                blk = w[:, H:].reshape(H, KT2, P)
                w[:, H:] = blk[:, :, ::-1].reshape(H, H)
            maps.append({"u": u, "v": v, "de": de, "do": do})
        return maps
    if mode == "bfly":
        n2 = np.arange(H, dtype=np.float64)[:, None]
        j2 = np.arange(H, dtype=np.float64)[None, :]
        de = _round_f32r(np.cos(np.pi * (2 * n2 + 1) * j2 / N).astype(np.float32))
        do = _round_f32r(
            np.cos(np.pi * (2 * n2 + 1) * (2 * j2 + 1) / (2 * N)).astype(
                np.float32
            )
        )
        maps = []
        for i in range(B):
            xi = np.asarray(x[i], dtype=np.float32)
            xr = xi[::-1]
            u = _round_f32r(xi[:H] + xr[:H])
            v = _round_f32r(xi[:H] - xr[:H])
            # Column-reverse blocks 8..15: pass-1 output partitions for those
            # blocks then come out mirror-ordered, which makes the pass-2
            # bottom-half fold a plain ascending read (see _build_bfly).
            for w in (u, v):
                blk = w[:, H:].reshape(H, KT2, P)
                w[:, H:] = blk[:, :, ::-1].reshape(H, H)
            maps.append({"u": u, "v": v, "de": de, "do": do})
        return maps
    dh, dl = _split_bf16(_dct_matrix_d())
    maps = []
    for i in range(B):
        xh, xl = _split_bf16(np.ascontiguousarray(x[i], dtype=np.float32))
        maps.append({"xh": xh, "xl": xl, "dh": dh, "dl": dl})
    return maps


def kernel(x: np.ndarray) -> np.ndarray:
    x = np.asarray(x)
    assert x.shape == (B, N, N), x.shape
    nc = _get_program(MODE)
    in_maps = _make_in_maps(x, MODE)
    res = run_bass_kernel_spmd(nc, in_maps, list(range(B)))
    out = np.stack([res.results[i]["z"] for i in range(B)], axis=0)
    return out.astype(np.float32, copy=False)

